# revision 1
# baseline (speedup 1.0000x reference)
"""Trainium2 Bass kernel for causal self-attention with GQA + RoPE.

Model: B=2, T=2048, C=2048, H=16 query heads, H_KV=4 kv heads, D=128.

Sharding (8 NeuronCores, pure SPMD, no collectives):
  core i -> batch b = i // 4, kv-group g = i % 4
            (query heads 4g..4g+3, kv head g, all T positions of batch b).
  Every core runs an identical program; only input data differs.
  o_proj is computed against the row-slice wo[512g:512(g+1), :], giving a
  partial [T, C] output per core; the sum over the 4 cores of each batch
  (the tensor-parallel all-reduce) is done on the host in numpy.

Device program per core (all matmuls fp32r = full PE rate at N>=256):
  - activations kept transposed: Q^T/K^T are [D, T] (D on partitions), which
    is what both the projection matmuls and the S^T = K @ Q^T matmuls want.
  - V is produced as V^T [D, T] then PE-transposed into natural [T, D] tiles
    (lhsT for the PV matmul).
  - RoPE: rotate_half is the linear map R, applied as a PE matmul
    (lhsT = R^T), then q_rope = q * cos + (R q) * sin on the vector engine.
  - causal flash-style attention without row-max (logits are provably small
    for this problem: |s| < ~6, exp never overflows), q in 512-wide chunks:
       S^T[k, q] 512-wide subtiles -> exp(scale*s) on ACT (psum -> sbuf)
       -> causal mask multiply on the 4 diagonal subtiles (host masks)
       -> y^T accumulated via lhsT=V tiles, rowsum broadcast via lhsT=ones
       -> 1/rowsum via ACT ln + exp(-x) (same table set as the softmax exp)
       -> y = y * rinv on DVE.
  - o_proj is interleaved per 512-row chunk so it overlaps the attention
    tail; wo/wq/wk/wv are streamed in per-chunk DMAs so the first matmul
    starts early.
"""

import math
import os

import numpy as np

os.environ.setdefault("MYCRO_LOCAL_CACHE", "1")

P = 128
D = 128
H = 16
H_KV = 4
GQ = H // H_KV  # 4 query heads per kv head (= per core)
B = 2
T_FULL = 2048
C_DIM = 2048
NCORES = 8
ROPE_BASE = 10000.0


def _rope_tables(T):
    inv_freq = 1.0 / (ROPE_BASE ** (np.arange(0, D, 2, dtype=np.float32) / D))
    t = np.arange(T, dtype=np.float32)
    freqs = np.outer(t, inv_freq)  # [T, D/2]
    emb = np.concatenate((freqs, freqs), axis=-1)  # [T, D]
    return (
        np.ascontiguousarray(np.cos(emb).T.astype(np.float32)),  # [D, T]
        np.ascontiguousarray(np.sin(emb).T.astype(np.float32)),
    )


def _rot_lhsT():
    # rotate_half(q) = R @ q with R[d, d+64] = -1 (d < 64), R[d, d-64] = +1.
    # matmul computes lhsT.T @ rhs, so pass lhsT = R^T.
    R = np.zeros((D, D), dtype=np.float32)
    half = D // 2
    R[np.arange(half), np.arange(half) + half] = -1.0
    R[np.arange(half) + half, np.arange(half)] = 1.0
    return np.ascontiguousarray(R.T)


def _mask4():
    # mask4[m][k, q] = 1 if (128*m + k) <= q else 0, for the 4 diagonal
    # k-subtiles of a 512-wide q chunk (S^T layout: k on partitions).
    m4 = np.zeros((4, P, 512), dtype=np.float32)
    q = np.arange(512)
    k = np.arange(P)
    for m in range(4):
        m4[m] = ((128 * m + k)[:, None] <= q[None, :]).astype(np.float32)
    return m4


def build_nc(T=T_FULL):
    """Build the per-core Bass/Tile program (identical across cores)."""
    from contextlib import ExitStack

    import concourse.mybir as mybir
    import concourse.tile as tile
    from concourse import bacc
    from concourse.masks import make_identity

    f32 = mybir.dt.float32
    f32r = mybir.dt.float32r
    Exp = mybir.ActivationFunctionType.Exp
    Ln = mybir.ActivationFunctionType.Ln
    MULT = mybir.AluOpType.mult
    ADD = mybir.AluOpType.add
    SCALE = 1.0 / math.sqrt(D)

    NCC = C_DIM // P  # 16 contraction chunks
    NQC = T // 512  # projection / attention q-chunks (512-wide)
    NCT = C_DIM // 512  # o_proj column tiles
    NKB = T // P  # 128-wide k subtiles
    XG = 4  # xt c-chunks per streamed tile

    nc = bacc.Bacc(
        "TRN2",
        target_bir_lowering=False,
        debug=False,
        num_devices=NCORES,
    )

    xt = nc.dram_tensor("xt", [C_DIM, T], f32r, kind="ExternalInput").ap()
    wq = nc.dram_tensor("wq", [C_DIM, GQ * D], f32r, kind="ExternalInput").ap()
    wk = nc.dram_tensor("wk", [C_DIM, D], f32r, kind="ExternalInput").ap()
    wv = nc.dram_tensor("wv", [C_DIM, D], f32r, kind="ExternalInput").ap()
    wo = nc.dram_tensor("wo", [GQ * D, C_DIM], f32r, kind="ExternalInput").ap()
    cosT = nc.dram_tensor("cosT", [D, T], f32, kind="ExternalInput").ap()
    sinT = nc.dram_tensor("sinT", [D, T], f32, kind="ExternalInput").ap()
    mask4 = nc.dram_tensor("mask4", [4, P, 512], f32, kind="ExternalInput").ap()
    onesm = nc.dram_tensor("onesm", [P, P], f32r, kind="ExternalInput").ap()
    rotm = nc.dram_tensor("rotm", [P, P], f32r, kind="ExternalInput").ap()
    out = nc.dram_tensor("out", [T, C_DIM], f32, kind="ExternalOutput").ap()

    with tile.TileContext(nc) as tc, ExitStack() as ctx:
        const = ctx.enter_context(tc.tile_pool(name="const", bufs=1))
        acts = ctx.enter_context(tc.tile_pool(name="acts", bufs=1))

        wq_r = wq.rearrange("(cc p) n -> p cc n", p=P)
        wk_r = wk.rearrange("(cc p) n -> p cc n", p=P)
        wv_r = wv.rearrange("(cc p) n -> p cc n", p=P)
        xt_r = xt.rearrange("(cc p) t -> p cc t", p=P)
        wo_r = wo.rearrange("(h p) (ct n) -> p h ct n", p=P, n=512)

        ones_sb = const.tile([P, P], f32r)
        rot_sb = const.tile([P, P], f32r)
        ident = const.tile([P, P], f32)
        mask_sb = const.tile([P, 4, 512], f32)

        # long-lived activations
        qt_sb = [acts.tile([P, T], f32r, name=f"qt{h}") for h in range(GQ)]
        kt_sb = acts.tile([P, T], f32r, name="kt")
        v_sb = acts.tile([P, NKB, D], f32r, name="vnat")
        y_sb = [acts.tile([P, T], f32r, name=f"yt{h}") for h in range(GQ)]

        # ---------------- phase 1: projections + rope ----------------
        with (
            tc.tile_pool(name="pwts", bufs=1) as wpool,
            tc.tile_pool(name="xts", bufs=4) as xt_pool,
            tc.tile_pool(name="rope_t", bufs=1) as rope_pool,
            tc.tile_pool(name="proj_ps", bufs=1, space="PSUM") as proj_ps,
            tc.tile_pool(name="aux_ps", bufs=1, space="PSUM") as aux_ps,
            tc.tile_pool(name="ptmp", bufs=2) as ptmp,
        ):
            # weight tiles: per-cc DMAs so the first projection matmul can
            # start as soon as chunk 0 lands (one 4MB DMA would stall ~30us).
            # xt streams on the sync queue; wq on scalar so neither blocks
            # the other.
            wq_sb = wpool.tile([P, NCC, GQ * D], f32r)
            wk_sb = wpool.tile([P, NCC, D], f32r)
            wv_sb = wpool.tile([P, NCC, D], f32r)
            lead_xs = []
            for xg in range(2):
                xs = xt_pool.tile([P, XG, 512], f32r, tag="xt", name=f"xs_l{xg}")
                nc.sync.dma_start(xs[:], xt_r[:, xg * XG : (xg + 1) * XG, 0:512])
                lead_xs.append(xs)
            for cc in range(NCC):
                nc.scalar.dma_start(wq_sb[:, cc, :], wq_r[:, cc, :])
                nc.gpsimd.dma_start(wk_sb[:, cc, :], wk_r[:, cc, :])
                nc.gpsimd.dma_start(wv_sb[:, cc, :], wv_r[:, cc, :])
            nc.gpsimd.dma_start(ones_sb[:], onesm)
            nc.gpsimd.dma_start(rot_sb[:], rotm)
            make_identity(nc, ident)
            cos_sb = rope_pool.tile([P, T], f32)
            nc.gpsimd.dma_start(cos_sb[:], cosT)
            sin_sb = rope_pool.tile([P, T], f32)
            nc.gpsimd.dma_start(sin_sb[:], sinT)
            for m in range(4):
                nc.gpsimd.dma_start(mask_sb[:, m, :], mask4[m])
            for qc in range(NQC):
                q0 = qc * 512
                xt_tiles = []
                for xg in range(NCC // XG):
                    if qc == 0 and xg < 2:
                        xt_tiles.append(lead_xs[xg])
                        continue
                    xs = xt_pool.tile([P, XG, 512], f32r, tag="xt")
                    nc.sync.dma_start(
                        xs[:], xt_r[:, xg * XG : (xg + 1) * XG, q0 : q0 + 512]
                    )
                    xt_tiles.append(xs)

                qp = [
                    proj_ps.tile([P, 512], f32, name=f"qp{h}", tag=f"qp{h}")
                    for h in range(GQ)
                ]
                kp = proj_ps.tile([P, 512], f32, name="kp", tag="kp")
                vp = proj_ps.tile([P, 512], f32, name="vp", tag="vp")
                for cc in range(NCC):
                    xtile = xt_tiles[cc // XG][:, cc % XG, :]
                    first, last = cc == 0, cc == NCC - 1
                    for h in range(GQ):
                        nc.tensor.matmul(
                            qp[h][:],
                            wq_sb[:, cc, h * D : (h + 1) * D],
                            xtile,
                            start=first,
                            stop=last,
                        )
                    nc.tensor.matmul(
                        kp[:], wk_sb[:, cc, :], xtile, start=first, stop=last
                    )
                    nc.tensor.matmul(
                        vp[:], wv_sb[:, cc, :], xtile, start=first, stop=last
                    )

                cosq = cos_sb[:, q0 : q0 + 512]
                sinq = sin_sb[:, q0 : q0 + 512]

                def rope(pt_ps, dst):
                    # dst = pt*cos + (R pt)*sin ; pt_ps is the PSUM projection
                    raw = ptmp.tile([P, 512], f32r, name="rraw", tag="rraw")
                    nc.scalar.copy(raw[:], pt_ps[:])
                    rp = aux_ps.tile([P, 512], f32, name="rotp", tag="rotp")
                    nc.tensor.matmul(rp[:], rot_sb[:], raw[:], start=True, stop=True)
                    nc.vector.tensor_tensor(dst, raw[:], cosq, MULT)
                    t2 = ptmp.tile([P, 512], f32, name="rt2", tag="rt2")
                    nc.vector.tensor_tensor(t2[:], rp[:], sinq, MULT)
                    nc.vector.tensor_tensor(dst, dst, t2[:], ADD)

                for h in range(GQ):
                    rope(qp[h], qt_sb[h][:, q0 : q0 + 512])
                rope(kp, kt_sb[:, q0 : q0 + 512])

                # V: evacuate V^T, then PE-transpose to natural [k, D] tiles
                vraw = ptmp.tile([P, 512], f32, name="vraw", tag="vraw")
                nc.scalar.copy(vraw[:], vp[:])
                for ks in range(4):
                    tp = aux_ps.tile([P, P], f32, name="vtrp", tag="vtrp")
                    nc.tensor.transpose(tp[:], vraw[:, ks * P : (ks + 1) * P], ident[:])
                    nc.vector.tensor_copy(v_sb[:, qc * 4 + ks, :], tp[:])

        # -------- phase 2: causal attention + interleaved o_proj --------
        with (
            tc.tile_pool(name="pt_pool", bufs=3) as pt_pool,
            tc.tile_pool(name="s_ps", bufs=2, space="PSUM") as s_ps,
            tc.tile_pool(name="y_ps", bufs=2, space="PSUM") as y_ps,
            tc.tile_pool(name="rs_ps", bufs=1, space="PSUM") as rs_ps,
            tc.tile_pool(name="o_ps", bufs=1, space="PSUM") as o_ps,
            tc.tile_pool(name="nrm", bufs=2) as nrm_pool,
            tc.tile_pool(name="ost", bufs=6) as ost_pool,
            tc.tile_pool(name="wot", bufs=2) as wot_pool,
        ):
            for aq in range(NQC):
                q0 = aq * 512
                nks = 4 * aq + 4  # number of 128-wide k subtiles (incl diag 4)
                for h in range(GQ):
                    qrhs = qt_sb[h][:, q0 : q0 + 512]
                    yp = y_ps.tile([P, 512], f32, name="yp", tag="yp")
                    rp_ = rs_ps.tile([P, 512], f32, name="rsp", tag="rsp")
                    for g in range(nks // 2):
                        subs = (2 * g, 2 * g + 1)
                        sp = s_ps.tile([P, 1024], f32, name="sp", tag="sp")
                        pt = pt_pool.tile([P, 1024], f32r, name="ptile", tag="ptile")
                        for j, ks in enumerate(subs):
                            nc.tensor.matmul(
                                sp[:, j * 512 : (j + 1) * 512],
                                kt_sb[:, ks * P : (ks + 1) * P],
                                qrhs,
                                start=True,
                                stop=True,
                            )
                        nc.scalar.activation(pt[:], sp[:], Exp, scale=SCALE)
                        for j, ks in enumerate(subs):
                            m = ks - (nks - 4)  # diagonal subtile index 0..3
                            if m >= 0:
                                w = 128 * (m + 1)
                                sl = pt[:, j * 512 : j * 512 + w]
                                nc.vector.tensor_tensor(
                                    sl, sl, mask_sb[:, m, :w], MULT
                                )
                        for j, ks in enumerate(subs):
                            first, last = ks == 0, ks == nks - 1
                            prhs = pt[:, j * 512 : (j + 1) * 512]
                            nc.tensor.matmul(
                                yp[:], v_sb[:, ks, :], prhs, start=first, stop=last
                            )
                            nc.tensor.matmul(
                                rp_[:], ones_sb[:], prhs, start=first, stop=last
                            )
                    # 1/rowsum: single custom-DVE op (~18 bits, plenty
                    # above the fp32r matmul noise floor; rowsum >= 1 so no
                    # edge cases). ACT Reciprocal/Ln would thrash the
                    # activation table sets against the softmax Exp.
                    rinv = nrm_pool.tile([P, 512], f32, name="rinv", tag="rinv")
                    nc.vector.reciprocal_approx_fast(rinv[:], rp_[:])
                    nc.vector.tensor_tensor(
                        y_sb[h][:, q0 : q0 + 512], yp[:], rinv[:], MULT
                    )
                # o_proj for this 512-row chunk (all 4 heads' y ready);
                # wo streamed per (aq, ct) and reused across the 4 q-blocks
                for ct in range(NCT):
                    wot = wot_pool.tile([P, GQ, 512], f32r, name="wot", tag="wot")
                    for h in range(GQ):
                        nc.sync.dma_start(wot[:, h, :], wo_r[:, h, ct, :])
                    for qb in range(4 * aq, 4 * aq + 4):
                        op = o_ps.tile([P, 512], f32, name="op", tag="op")
                        for h in range(GQ):
                            nc.tensor.matmul(
                                op[:],
                                y_sb[h][:, qb * P : (qb + 1) * P],
                                wot[:, h, :],
                                start=(h == 0),
                                stop=(h == GQ - 1),
                            )
                        ot = ost_pool.tile([P, 512], f32, name="ot", tag="ot")
                        nc.vector.tensor_copy(ot[:], op[:])
                        oq = nc.gpsimd if (ct % 2 == 0) else nc.scalar
                        oq.dma_start(
                            out[qb * P : (qb + 1) * P, ct * 512 : (ct + 1) * 512],
                            ot[:],
                        )

    nc.compile()
    return nc


def make_in_maps(x, wq, wk, wv, wo, T=T_FULL):
    """Per-core input dicts for run_bass_kernel_spmd."""
    cosT, sinT = _rope_tables(T)
    m4 = _mask4()
    onesm = np.ones((P, P), dtype=np.float32)
    rotm = _rot_lhsT()

    xts = [np.ascontiguousarray(x[b].T.astype(np.float32)) for b in range(B)]
    in_maps = []
    for core in range(NCORES):
        b, g = core // 4, core % 4
        in_maps.append(
            {
                "xt": xts[b],
                "wq": np.ascontiguousarray(wq[:, 512 * g : 512 * (g + 1)]),
                "wk": np.ascontiguousarray(wk[:, D * g : D * (g + 1)]),
                "wv": np.ascontiguousarray(wv[:, D * g : D * (g + 1)]),
                "wo": np.ascontiguousarray(wo[512 * g : 512 * (g + 1), :]),
                "cosT": cosT,
                "sinT": sinT,
                "mask4": m4,
                "onesm": onesm,
                "rotm": rotm,
            }
        )
    return in_maps


_NC_CACHE = {}


def _get_nc(T=T_FULL):
    if T not in _NC_CACHE:
        _NC_CACHE[T] = build_nc(T)
    return _NC_CACHE[T]


def run(inputs, trace=False):
    """Run on 8 NeuronCores. Returns (full_output, BassKernelResults)."""
    from concourse.bass_utils import run_bass_kernel_spmd

    x = np.asarray(inputs["x"], dtype=np.float32)
    in_maps = make_in_maps(
        x,
        np.asarray(inputs["wq"], dtype=np.float32),
        np.asarray(inputs["wk"], dtype=np.float32),
        np.asarray(inputs["wv"], dtype=np.float32),
        np.asarray(inputs["wo"], dtype=np.float32),
    )
    nc = _get_nc()
    res = run_bass_kernel_spmd(nc, in_maps, list(range(NCORES)), trace=trace)
    outs = res.results
    full = np.zeros((B, T_FULL, C_DIM), dtype=np.float32)
    for core in range(NCORES):
        full[core // 4] += outs[core]["out"]
    return full, res


def kernel(**inputs):
    full, _ = run(inputs, trace=False)
    return full



# revision 2
# speedup vs baseline: 1.1317x; 1.1317x over previous
"""Trainium2 Bass kernel for causal self-attention with GQA + RoPE.

Model: B=2, T=2048, C=2048, H=16 query heads, H_KV=4 kv heads, D=128.

Sharding (8 NeuronCores, pure SPMD, no collectives):
  core i -> batch b = i // 4, kv-group g = i % 4
            (query heads 4g..4g+3, kv head g, all T positions of batch b).
  Every core runs an identical program; only input data differs.
  o_proj is computed against the row-slice wo[512g:512(g+1), :], giving a
  partial [T, C] output per core; the sum over the 4 cores of each batch
  (the tensor-parallel all-reduce) is done on the host in numpy.

Device program per core (all matmuls fp32r = full PE rate at N>=256):
  - activations kept transposed: Q^T/K^T are [D, T] (D on partitions), which
    is what both the projection matmuls and the S^T = K @ Q^T matmuls want.
  - V is produced as V^T [D, T] then PE-transposed into natural [T, D] tiles
    (lhsT for the PV matmul).
  - RoPE: rotate_half is the linear map R, applied as a PE matmul
    (lhsT = R^T), then q_rope = q * cos + (R q) * sin on the vector engine.
  - causal flash-style attention without row-max (logits are provably small
    for this problem: |s| < ~6, exp never overflows), q in 512-wide chunks:
       S^T[k, q] 512-wide subtiles -> exp(scale*s) on ACT (psum -> sbuf)
       -> causal mask multiply on the 4 diagonal subtiles (host masks)
       -> y^T accumulated via lhsT=V tiles
       -> rowsum: DVE pair-sums adjacent exp subtiles, then a single
          ones-lhsT matmul accumulates the pairs in PSUM (half the PE rows
          of the naive per-subtile ones matmul)
       -> 1/rowsum via DVE reciprocal_approx_fast, y = y * rinv on DVE.
  - o_proj per 512-row chunk; wo is loaded ONCE at phase-2 start and kept
    resident (32KB/partition) instead of being re-streamed per chunk.
  - xt streams in [P, 2, 512] tiles with bufs=8 so the next chunk's DMA
    issues ~20us ahead of use (the bufs=4 whole-chunk scheme stalled all
    engines ~5-7us at every chunk boundary).
  - o_proj PSUM evacuation on the ACT engine (DVE was near-critical);
    o_proj output tiles and rowsum tiles share a 2-buffer PSUM rotation.
"""

import math
import os

import numpy as np

os.environ.setdefault("MYCRO_LOCAL_CACHE", "1")

P = 128
D = 128
H = 16
H_KV = 4
GQ = H // H_KV  # 4 query heads per kv head (= per core)
B = 2
T_FULL = 2048
C_DIM = 2048
NCORES = 8
ROPE_BASE = 10000.0


def _rope_tables(T):
    inv_freq = 1.0 / (ROPE_BASE ** (np.arange(0, D, 2, dtype=np.float32) / D))
    t = np.arange(T, dtype=np.float32)
    freqs = np.outer(t, inv_freq)  # [T, D/2]
    emb = np.concatenate((freqs, freqs), axis=-1)  # [T, D]
    return (
        np.ascontiguousarray(np.cos(emb).T.astype(np.float32)),  # [D, T]
        np.ascontiguousarray(np.sin(emb).T.astype(np.float32)),
    )


def _rot_lhsT():
    # rotate_half(q) = R @ q with R[d, d+64] = -1 (d < 64), R[d, d-64] = +1.
    # matmul computes lhsT.T @ rhs, so pass lhsT = R^T.
    R = np.zeros((D, D), dtype=np.float32)
    half = D // 2
    R[np.arange(half), np.arange(half) + half] = -1.0
    R[np.arange(half) + half, np.arange(half)] = 1.0
    return np.ascontiguousarray(R.T)


def _mask4():
    # mask4[m][k, q] = 1 if (128*m + k) <= q else 0, for the 4 diagonal
    # k-subtiles of a 512-wide q chunk (S^T layout: k on partitions).
    m4 = np.zeros((4, P, 512), dtype=np.float32)
    q = np.arange(512)
    k = np.arange(P)
    for m in range(4):
        m4[m] = ((128 * m + k)[:, None] <= q[None, :]).astype(np.float32)
    return m4


def build_nc(T=T_FULL):
    """Build the per-core Bass/Tile program (identical across cores)."""
    from contextlib import ExitStack

    import concourse.mybir as mybir
    import concourse.tile as tile
    from concourse import bacc
    from concourse.masks import make_identity

    f32 = mybir.dt.float32
    f32r = mybir.dt.float32r
    Exp = mybir.ActivationFunctionType.Exp
    MULT = mybir.AluOpType.mult
    ADD = mybir.AluOpType.add
    SCALE = 1.0 / math.sqrt(D)

    NCC = C_DIM // P  # 16 contraction chunks
    NQC = T // 512  # projection / attention q-chunks (512-wide)
    NCT = C_DIM // 512  # o_proj column tiles
    NKB = T // P  # 128-wide k subtiles
    XG = 2  # xt c-chunks per streamed tile

    nc = bacc.Bacc(
        "TRN2",
        target_bir_lowering=False,
        debug=False,
        num_devices=NCORES,
    )

    xt = nc.dram_tensor("xt", [C_DIM, T], f32r, kind="ExternalInput").ap()
    wq = nc.dram_tensor("wq", [C_DIM, GQ * D], f32r, kind="ExternalInput").ap()
    wk = nc.dram_tensor("wk", [C_DIM, D], f32r, kind="ExternalInput").ap()
    wv = nc.dram_tensor("wv", [C_DIM, D], f32r, kind="ExternalInput").ap()
    wo = nc.dram_tensor("wo", [GQ * D, C_DIM], f32r, kind="ExternalInput").ap()
    cosT = nc.dram_tensor("cosT", [D, T], f32, kind="ExternalInput").ap()
    sinT = nc.dram_tensor("sinT", [D, T], f32, kind="ExternalInput").ap()
    mask4 = nc.dram_tensor("mask4", [4, P, 512], f32, kind="ExternalInput").ap()
    onesm = nc.dram_tensor("onesm", [P, P], f32r, kind="ExternalInput").ap()
    rotm = nc.dram_tensor("rotm", [P, P], f32r, kind="ExternalInput").ap()
    out = nc.dram_tensor("out", [T, C_DIM], f32, kind="ExternalOutput").ap()

    with tile.TileContext(nc) as tc, ExitStack() as ctx:
        const = ctx.enter_context(tc.tile_pool(name="const", bufs=1))
        acts = ctx.enter_context(tc.tile_pool(name="acts", bufs=1))

        wq_r = wq.rearrange("(cc p) n -> p cc n", p=P)
        wk_r = wk.rearrange("(cc p) n -> p cc n", p=P)
        wv_r = wv.rearrange("(cc p) n -> p cc n", p=P)
        xt_r = xt.rearrange("(cc p) t -> p cc t", p=P)
        wo_r = wo.rearrange("(h p) (ct n) -> p h ct n", p=P, n=512)

        ones_sb = const.tile([P, P], f32r)
        rot_sb = const.tile([P, P], f32r)
        ident = const.tile([P, P], f32)
        mask_sb = const.tile([P, 4, 512], f32)

        # long-lived activations
        qt_sb = [acts.tile([P, T], f32r, name=f"qt{h}") for h in range(GQ)]
        kt_sb = acts.tile([P, T], f32r, name="kt")
        v_sb = acts.tile([P, NKB, D], f32r, name="vnat")
        y_sb = [acts.tile([P, T], f32r, name=f"yt{h}") for h in range(GQ)]

        # ---------------- phase 1: projections + rope ----------------
        NXT = NCC // XG  # xt tiles per q-chunk
        with (
            tc.tile_pool(name="pwts", bufs=1) as wpool,
            tc.tile_pool(name="xts", bufs=8) as xt_pool,
            tc.tile_pool(name="rope_t", bufs=1) as rope_pool,
            tc.tile_pool(name="proj_ps", bufs=1, space="PSUM") as proj_ps,
            tc.tile_pool(name="aux_ps", bufs=1, space="PSUM") as aux_ps,
            tc.tile_pool(name="ptmp", bufs=2) as ptmp,
        ):
            # weight tiles: per-cc DMAs so the first projection matmul can
            # start as soon as chunk 0 lands. xt streams on the sync queue;
            # wq on scalar; wk/wv + consts on gpsimd.
            wq_sb = wpool.tile([P, NCC, GQ * D], f32r)
            wk_sb = wpool.tile([P, NCC, D], f32r)
            wv_sb = wpool.tile([P, NCC, D], f32r)
            # first-needed first: wq/wk/wv chunk 0, lead xt tiles
            nc.scalar.dma_start(wq_sb[:, 0, :], wq_r[:, 0, :])
            nc.gpsimd.dma_start(wk_sb[:, 0, :], wk_r[:, 0, :])
            nc.gpsimd.dma_start(wv_sb[:, 0, :], wv_r[:, 0, :])
            lead_xs = []
            for xg in range(3):
                xs = xt_pool.tile([P, XG, 512], f32r, tag="xt", name=f"xs_l{xg}")
                nc.sync.dma_start(xs[:], xt_r[:, xg * XG : (xg + 1) * XG, 0:512])
                lead_xs.append(xs)
            for cc in range(1, NCC):
                nc.scalar.dma_start(wq_sb[:, cc, :], wq_r[:, cc, :])
                nc.gpsimd.dma_start(wk_sb[:, cc, :], wk_r[:, cc, :])
                nc.gpsimd.dma_start(wv_sb[:, cc, :], wv_r[:, cc, :])
            nc.gpsimd.dma_start(rot_sb[:], rotm)
            make_identity(nc, ident)
            cos_sb = rope_pool.tile([P, T], f32)
            nc.gpsimd.dma_start(cos_sb[:], cosT)
            sin_sb = rope_pool.tile([P, T], f32)
            nc.gpsimd.dma_start(sin_sb[:], sinT)
            nc.gpsimd.dma_start(ones_sb[:], onesm)
            for m in range(4):
                nc.gpsimd.dma_start(mask_sb[:, m, :], mask4[m])
            for qc in range(NQC):
                q0 = qc * 512
                xt_tiles = []
                for xg in range(NXT):
                    if qc == 0 and xg < 3:
                        xt_tiles.append(lead_xs[xg])
                        continue
                    xs = xt_pool.tile([P, XG, 512], f32r, tag="xt")
                    nc.sync.dma_start(
                        xs[:], xt_r[:, xg * XG : (xg + 1) * XG, q0 : q0 + 512]
                    )
                    xt_tiles.append(xs)

                qp = [
                    proj_ps.tile([P, 512], f32, name=f"qp{h}", tag=f"qp{h}")
                    for h in range(GQ)
                ]
                kp = proj_ps.tile([P, 512], f32, name="kp", tag="kp")
                vp = proj_ps.tile([P, 512], f32, name="vp", tag="vp")
                for cc in range(NCC):
                    xtile = xt_tiles[cc // XG][:, cc % XG, :]
                    first, last = cc == 0, cc == NCC - 1
                    for h in range(GQ):
                        nc.tensor.matmul(
                            qp[h][:],
                            wq_sb[:, cc, h * D : (h + 1) * D],
                            xtile,
                            start=first,
                            stop=last,
                        )
                    nc.tensor.matmul(
                        kp[:], wk_sb[:, cc, :], xtile, start=first, stop=last
                    )
                    nc.tensor.matmul(
                        vp[:], wv_sb[:, cc, :], xtile, start=first, stop=last
                    )

                cosq = cos_sb[:, q0 : q0 + 512]
                sinq = sin_sb[:, q0 : q0 + 512]

                def rope(pt_ps, dst):
                    # dst = pt*cos + (R pt)*sin ; pt_ps is the PSUM projection
                    raw = ptmp.tile([P, 512], f32r, name="rraw", tag="rraw")
                    nc.scalar.copy(raw[:], pt_ps[:])
                    rp = aux_ps.tile([P, 512], f32, name="rotp", tag="rotp")
                    nc.tensor.matmul(rp[:], rot_sb[:], raw[:], start=True, stop=True)
                    nc.vector.tensor_tensor(dst, raw[:], cosq, MULT)
                    t2 = ptmp.tile([P, 512], f32, name="rt2", tag="rt2")
                    nc.vector.tensor_tensor(t2[:], rp[:], sinq, MULT)
                    nc.vector.tensor_tensor(dst, dst, t2[:], ADD)

                for h in range(GQ):
                    rope(qp[h], qt_sb[h][:, q0 : q0 + 512])
                rope(kp, kt_sb[:, q0 : q0 + 512])

                # V: evacuate V^T, then PE-transpose to natural [k, D] tiles
                vraw = ptmp.tile([P, 512], f32, name="vraw", tag="vraw")
                nc.scalar.copy(vraw[:], vp[:])
                for ks in range(4):
                    tp = aux_ps.tile([P, P], f32, name="vtrp", tag="vtrp")
                    nc.tensor.transpose(tp[:], vraw[:, ks * P : (ks + 1) * P], ident[:])
                    nc.vector.tensor_copy(v_sb[:, qc * 4 + ks, :], tp[:])

        # -------- phase 2: causal attention + interleaved o_proj --------
        with (
            tc.tile_pool(name="wo_pool", bufs=1) as wo_pool,
            tc.tile_pool(name="pt_pool", bufs=3) as pt_pool,
            tc.tile_pool(name="pair_pool", bufs=3) as pair_pool,
            tc.tile_pool(name="s_ps", bufs=2, space="PSUM") as s_ps,
            tc.tile_pool(name="y_ps", bufs=2, space="PSUM") as y_ps,
            tc.tile_pool(name="ro_ps", bufs=2, space="PSUM") as ro_ps,
            tc.tile_pool(name="nrm", bufs=2) as nrm_pool,
            tc.tile_pool(name="ost", bufs=4) as ost_pool,
        ):
            # wo resident for all of phase 2 (32KB/partition); the per-slice
            # DMAs land during attention chunk 0 (subtile deps let o_proj
            # start as soon as its ct slice is in).
            wo_sb = wo_pool.tile([P, GQ, NCT, 512], f32r)
            for ct in range(NCT):
                for h in range(GQ):
                    q = nc.sync if (h % 2 == 0) else nc.gpsimd
                    q.dma_start(wo_sb[:, h, ct, :], wo_r[:, h, ct, :])
            for aq in range(NQC):
                q0 = aq * 512
                nks = 4 * aq + 4  # number of 128-wide k subtiles (incl diag 4)
                npair = nks // 2
                for h in range(GQ):
                    qrhs = qt_sb[h][:, q0 : q0 + 512]
                    yp = y_ps.tile([P, 512], f32, name="yp", tag="yp")
                    rs = ro_ps.tile([P, 512], f32, name="rs", tag="ro")
                    for g in range(npair):
                        subs = (2 * g, 2 * g + 1)
                        sp = s_ps.tile([P, 1024], f32, name="sp", tag="sp")
                        pt = pt_pool.tile([P, 1024], f32r, name="ptile", tag="ptile")
                        for j, ks in enumerate(subs):
                            nc.tensor.matmul(
                                sp[:, j * 512 : (j + 1) * 512],
                                kt_sb[:, ks * P : (ks + 1) * P],
                                qrhs,
                                start=True,
                                stop=True,
                            )
                        nc.scalar.activation(pt[:], sp[:], Exp, scale=SCALE)
                        for j, ks in enumerate(subs):
                            m = ks - (nks - 4)  # diagonal subtile index 0..3
                            if m >= 0:
                                w = 128 * (m + 1)
                                sl = pt[:, j * 512 : j * 512 + w]
                                nc.vector.tensor_tensor(
                                    sl, sl, mask_sb[:, m, :w], MULT
                                )
                        for j, ks in enumerate(subs):
                            first, last = ks == 0, ks == nks - 1
                            prhs = pt[:, j * 512 : (j + 1) * 512]
                            nc.tensor.matmul(
                                yp[:], v_sb[:, ks, :], prhs, start=first, stop=last
                            )
                        # rowsum: pair-sum the two exp subtiles on DVE, then
                        # one ones-matmul per pair (half the PE rows).
                        pair = pair_pool.tile([P, 512], f32r, name="pair", tag="pair")
                        nc.vector.tensor_tensor(
                            pair[:], pt[:, 0:512], pt[:, 512:1024], ADD
                        )
                        nc.tensor.matmul(
                            rs[:],
                            ones_sb[:],
                            pair[:],
                            start=(g == 0),
                            stop=(g == npair - 1),
                        )
                    # 1/rowsum: single custom-DVE op (~18 bits, plenty
                    # above the fp32r matmul noise floor; rowsum >= 1 so no
                    # edge cases).
                    rinv = nrm_pool.tile([P, 512], f32, name="rinv", tag="rinv")
                    nc.vector.reciprocal_approx_fast(rinv[:], rs[:])
                    nc.vector.tensor_tensor(
                        y_sb[h][:, q0 : q0 + 512], yp[:], rinv[:], MULT
                    )
                # o_proj for this 512-row chunk (all 4 heads' y ready);
                # op tiles share the 2-buffer "ro" PSUM rotation with rs.
                for ct in range(NCT):
                    for qb in range(4 * aq, 4 * aq + 4):
                        op = ro_ps.tile([P, 512], f32, name="op", tag="ro")
                        for h in range(GQ):
                            nc.tensor.matmul(
                                op[:],
                                y_sb[h][:, qb * P : (qb + 1) * P],
                                wo_sb[:, h, ct, :],
                                start=(h == 0),
                                stop=(h == GQ - 1),
                            )
                        ot = ost_pool.tile([P, 512], f32, name="ot", tag="ot")
                        nc.scalar.copy(ot[:], op[:])
                        oq = nc.gpsimd if (ct % 2 == 0) else nc.scalar
                        oq.dma_start(
                            out[qb * P : (qb + 1) * P, ct * 512 : (ct + 1) * 512],
                            ot[:],
                        )

    nc.compile()
    return nc


def make_in_maps(x, wq, wk, wv, wo, T=T_FULL):
    """Per-core input dicts for run_bass_kernel_spmd."""
    cosT, sinT = _rope_tables(T)
    m4 = _mask4()
    onesm = np.ones((P, P), dtype=np.float32)
    rotm = _rot_lhsT()

    xts = [np.ascontiguousarray(x[b].T.astype(np.float32)) for b in range(B)]
    in_maps = []
    for core in range(NCORES):
        b, g = core // 4, core % 4
        in_maps.append(
            {
                "xt": xts[b],
                "wq": np.ascontiguousarray(wq[:, 512 * g : 512 * (g + 1)]),
                "wk": np.ascontiguousarray(wk[:, D * g : D * (g + 1)]),
                "wv": np.ascontiguousarray(wv[:, D * g : D * (g + 1)]),
                "wo": np.ascontiguousarray(wo[512 * g : 512 * (g + 1), :]),
                "cosT": cosT,
                "sinT": sinT,
                "mask4": m4,
                "onesm": onesm,
                "rotm": rotm,
            }
        )
    return in_maps


_NC_CACHE = {}


def _get_nc(T=T_FULL):
    if T not in _NC_CACHE:
        _NC_CACHE[T] = build_nc(T)
    return _NC_CACHE[T]


def run(inputs, trace=False):
    """Run on 8 NeuronCores. Returns (full_output, BassKernelResults)."""
    from concourse.bass_utils import run_bass_kernel_spmd

    x = np.asarray(inputs["x"], dtype=np.float32)
    in_maps = make_in_maps(
        x,
        np.asarray(inputs["wq"], dtype=np.float32),
        np.asarray(inputs["wk"], dtype=np.float32),
        np.asarray(inputs["wv"], dtype=np.float32),
        np.asarray(inputs["wo"], dtype=np.float32),
    )
    nc = _get_nc()
    res = run_bass_kernel_spmd(nc, in_maps, list(range(NCORES)), trace=trace)
    outs = res.results
    full = np.zeros((B, T_FULL, C_DIM), dtype=np.float32)
    for core in range(NCORES):
        full[core // 4] += outs[core]["out"]
    return full, res


def kernel(**inputs):
    full, _ = run(inputs, trace=False)
    return full


# revision 7
# speedup vs baseline: 1.1635x; 1.0281x over previous
"""Trainium2 Bass kernel for causal self-attention with GQA + RoPE.

Model: B=2, T=2048, C=2048, H=16 query heads, H_KV=4 kv heads, D=128.

Sharding (8 NeuronCores, pure SPMD, no collectives):
  core i -> batch b = i // 4, kv-group g = i % 4
            (query heads 4g..4g+3, kv head g, all T positions of batch b).
  Every core runs an identical program; only input data differs.
  o_proj is computed against the row-slice wo[512g:512(g+1), :], giving a
  partial [T, C] output per core; the sum over the 4 cores of each batch
  (the tensor-parallel all-reduce) is done on the host in numpy.

Device program per core (all matmuls fp32r = full PE rate at N>=256):
  - activations kept transposed: Q^T/K^T are [D, T] (D on partitions), which
    is what both the projection matmuls and the S^T = K @ Q^T matmuls want.
  - V is produced as V^T [D, T] then PE-transposed into natural [T, D] tiles
    (lhsT for the PV matmul).
  - RoPE: rotate_half is the linear map R, applied as a PE matmul
    (lhsT = R^T), then q_rope = q * cos + (R q) * sin on the vector engine.
  - causal flash-style attention without row-max (logits are provably small
    for this problem: |s| < ~6, exp never overflows), q in 512-wide chunks:
       S^T[k, q] 512-wide subtiles -> exp(scale*s) on ACT (psum -> sbuf)
       -> causal mask multiply on the 4 diagonal subtiles (host masks)
       -> y^T accumulated via lhsT=V tiles
       -> rowsum: DVE pair-sums adjacent exp subtiles, then a single
          ones-lhsT matmul accumulates the pairs in PSUM (half the PE rows
          of the naive per-subtile ones matmul)
       -> 1/rowsum via DVE reciprocal_approx_fast, y = y * rinv on DVE.
  - o_proj per 512-row chunk; wo is loaded ONCE at phase-2 start and kept
    resident (32KB/partition) instead of being re-streamed per chunk.
  - xt streams in [P, 2, 512] tiles with bufs=8 so the next chunk's DMA
    issues ~20us ahead of use (the bufs=4 whole-chunk scheme stalled all
    engines ~5-7us at every chunk boundary).
  - o_proj PSUM evacuation on the ACT engine (DVE was near-critical);
    o_proj output tiles and rowsum tiles share a 2-buffer PSUM rotation.
"""

import math
import os

import numpy as np

os.environ.setdefault("MYCRO_LOCAL_CACHE", "1")

P = 128
D = 128
H = 16
H_KV = 4
GQ = H // H_KV  # 4 query heads per kv head (= per core)
B = 2
T_FULL = 2048
C_DIM = 2048
NCORES = 8
ROPE_BASE = 10000.0


def _rope_tables(T):
    inv_freq = 1.0 / (ROPE_BASE ** (np.arange(0, D, 2, dtype=np.float32) / D))
    t = np.arange(T, dtype=np.float32)
    freqs = np.outer(t, inv_freq)  # [T, D/2]
    emb = np.concatenate((freqs, freqs), axis=-1)  # [T, D]
    return (
        np.ascontiguousarray(np.cos(emb).T.astype(np.float32)),  # [D, T]
        np.ascontiguousarray(np.sin(emb).T.astype(np.float32)),
    )


def _rot_lhsT():
    # rotate_half(q) = R @ q with R[d, d+64] = -1 (d < 64), R[d, d-64] = +1.
    # matmul computes lhsT.T @ rhs, so pass lhsT = R^T.
    R = np.zeros((D, D), dtype=np.float32)
    half = D // 2
    R[np.arange(half), np.arange(half) + half] = -1.0
    R[np.arange(half) + half, np.arange(half)] = 1.0
    return np.ascontiguousarray(R.T)


def _mask4():
    # mask4[m][k, q] = 1 if (128*m + k) <= q else 0, for the 4 diagonal
    # k-subtiles of a 512-wide q chunk (S^T layout: k on partitions).
    m4 = np.zeros((4, P, 512), dtype=np.float32)
    q = np.arange(512)
    k = np.arange(P)
    for m in range(4):
        m4[m] = ((128 * m + k)[:, None] <= q[None, :]).astype(np.float32)
    return m4


def build_nc(T=T_FULL):
    """Build the per-core Bass/Tile program (identical across cores)."""
    from contextlib import ExitStack

    import concourse.mybir as mybir
    import concourse.tile as tile
    from concourse import bacc
    from concourse.masks import make_identity

    f32 = mybir.dt.float32
    f32r = mybir.dt.float32r
    Exp = mybir.ActivationFunctionType.Exp
    MULT = mybir.AluOpType.mult
    ADD = mybir.AluOpType.add
    SCALE = 1.0 / math.sqrt(D)

    NCC = C_DIM // P  # 16 contraction chunks
    NQC = T // 512  # projection / attention q-chunks (512-wide)
    NCT = C_DIM // 512  # o_proj column tiles
    NKB = T // P  # 128-wide k subtiles
    XG = 2  # xt c-chunks per streamed tile

    nc = bacc.Bacc(
        "TRN2",
        target_bir_lowering=False,
        debug=False,
        num_devices=NCORES,
    )

    xt = nc.dram_tensor("xt", [C_DIM, T], f32r, kind="ExternalInput").ap()
    wq = nc.dram_tensor("wq", [C_DIM, GQ * D], f32r, kind="ExternalInput").ap()
    wk = nc.dram_tensor("wk", [C_DIM, D], f32r, kind="ExternalInput").ap()
    wv = nc.dram_tensor("wv", [C_DIM, D], f32r, kind="ExternalInput").ap()
    wo = nc.dram_tensor("wo", [GQ * D, C_DIM], f32r, kind="ExternalInput").ap()
    cosT = nc.dram_tensor("cosT", [D, T], f32, kind="ExternalInput").ap()
    sinT = nc.dram_tensor("sinT", [D, T], f32, kind="ExternalInput").ap()
    mask4 = nc.dram_tensor("mask4", [4, P, 512], f32, kind="ExternalInput").ap()
    onesm = nc.dram_tensor("onesm", [P, P], f32r, kind="ExternalInput").ap()
    rotm = nc.dram_tensor("rotm", [P, P], f32r, kind="ExternalInput").ap()
    out = nc.dram_tensor("out", [T, C_DIM], f32, kind="ExternalOutput").ap()

    with tile.TileContext(nc) as tc, ExitStack() as ctx:
        const = ctx.enter_context(tc.tile_pool(name="const", bufs=1))
        acts = ctx.enter_context(tc.tile_pool(name="acts", bufs=1))

        wq_r = wq.rearrange("(cc p) n -> p cc n", p=P)
        wk_r = wk.rearrange("(cc p) n -> p cc n", p=P)
        wv_r = wv.rearrange("(cc p) n -> p cc n", p=P)
        xt_r = xt.rearrange("(cc p) t -> p cc t", p=P)
        wo_r = wo.rearrange("(h p) (ct n) -> p h ct n", p=P, n=512)

        ones_sb = const.tile([P, P], f32r)
        rot_sb = const.tile([P, P], f32r)
        ident = const.tile([P, P], f32)
        mask_sb = const.tile([P, 4, 512], f32)

        # long-lived activations
        qt_sb = [acts.tile([P, T], f32r, name=f"qt{h}") for h in range(GQ)]
        kt_sb = acts.tile([P, T], f32r, name="kt")
        v_sb = acts.tile([P, NKB, D], f32r, name="vnat")
        y_sb = [acts.tile([P, T], f32r, name=f"yt{h}") for h in range(GQ)]

        # ---------------- phase 1: projections + rope ----------------
        NXT = NCC // XG  # xt tiles per q-chunk
        with (
            tc.tile_pool(name="pwts", bufs=1) as wpool,
            tc.tile_pool(name="xts", bufs=8) as xt_pool,
            tc.tile_pool(name="rope_t", bufs=1) as rope_pool,
            tc.tile_pool(name="proj_ps", bufs=1, space="PSUM") as proj_ps,
            tc.tile_pool(name="aux_ps", bufs=2, space="PSUM") as aux_ps,
            tc.tile_pool(name="ptmp", bufs=2) as ptmp,
        ):
            # weight tiles: per-cc DMAs so the first projection matmul can
            # start as soon as chunk 0 lands. xt streams on the sync queue;
            # wq on scalar; wk/wv + consts on gpsimd.
            wq_sb = wpool.tile([P, NCC, GQ * D], f32r)
            wk_sb = wpool.tile([P, NCC, D], f32r)
            wv_sb = wpool.tile([P, NCC, D], f32r)
            # first-needed first: wq/wk/wv chunk 0, lead xt tiles
            nc.scalar.dma_start(wq_sb[:, 0, :], wq_r[:, 0, :])
            nc.gpsimd.dma_start(wk_sb[:, 0, :], wk_r[:, 0, :])
            nc.gpsimd.dma_start(wv_sb[:, 0, :], wv_r[:, 0, :])
            lead_xs = []
            for xg in range(3):
                xs = xt_pool.tile([P, XG, 512], f32r, tag="xt", name=f"xs_l{xg}")
                nc.sync.dma_start(xs[:], xt_r[:, xg * XG : (xg + 1) * XG, 0:512])
                lead_xs.append(xs)
            nc.gpsimd.dma_start(rot_sb[:], rotm)
            # cos/sin on the fast sync HW queue, ahead of the chunk-1 xt
            # stream: on the gpsimd SW queue they landed ~60us in and stalled
            # the entire chunk-0 rope.
            cos_sb = rope_pool.tile([P, T], f32)
            nc.sync.dma_start(cos_sb[:], cosT)
            sin_sb = rope_pool.tile([P, T], f32)
            nc.sync.dma_start(sin_sb[:], sinT)
            for cc in range(1, NCC):
                nc.scalar.dma_start(wq_sb[:, cc, :], wq_r[:, cc, :])
                nc.gpsimd.dma_start(wk_sb[:, cc, :], wk_r[:, cc, :])
                nc.gpsimd.dma_start(wv_sb[:, cc, :], wv_r[:, cc, :])
            make_identity(nc, ident)
            nc.gpsimd.dma_start(ones_sb[:], onesm)
            for m in range(4):
                nc.gpsimd.dma_start(mask_sb[:, m, :], mask4[m])
            for qc in range(NQC):
                q0 = qc * 512
                xt_tiles = []
                for xg in range(NXT):
                    if qc == 0 and xg < 3:
                        xt_tiles.append(lead_xs[xg])
                        continue
                    xs = xt_pool.tile([P, XG, 512], f32r, tag="xt")
                    nc.sync.dma_start(
                        xs[:], xt_r[:, xg * XG : (xg + 1) * XG, q0 : q0 + 512]
                    )
                    xt_tiles.append(xs)

                qp = [
                    proj_ps.tile([P, 512], f32, name=f"qp{h}", tag=f"qp{h}")
                    for h in range(GQ)
                ]
                kp = proj_ps.tile([P, 512], f32, name="kp", tag="kp")
                vp = proj_ps.tile([P, 512], f32, name="vp", tag="vp")
                for cc in range(NCC):
                    xtile = xt_tiles[cc // XG][:, cc % XG, :]
                    first, last = cc == 0, cc == NCC - 1
                    for h in range(GQ):
                        nc.tensor.matmul(
                            qp[h][:],
                            wq_sb[:, cc, h * D : (h + 1) * D],
                            xtile,
                            start=first,
                            stop=last,
                        )
                    nc.tensor.matmul(
                        kp[:], wk_sb[:, cc, :], xtile, start=first, stop=last
                    )
                    nc.tensor.matmul(
                        vp[:], wv_sb[:, cc, :], xtile, start=first, stop=last
                    )

                cosq = cos_sb[:, q0 : q0 + 512]
                sinq = sin_sb[:, q0 : q0 + 512]

                def rope(pt_ps, dst):
                    # dst = pt*cos + (R pt)*sin ; pt_ps is the PSUM projection
                    raw = ptmp.tile([P, 512], f32r, name="rraw", tag="rraw", bufs=4)
                    nc.scalar.copy(raw[:], pt_ps[:])
                    rp = aux_ps.tile([P, 512], f32, name="rotp", tag="aux")
                    nc.tensor.matmul(rp[:], rot_sb[:], raw[:], start=True, stop=True)
                    nc.vector.tensor_tensor(dst, raw[:], cosq, MULT)
                    t2 = ptmp.tile([P, 512], f32, name="rt2", tag="rt2", bufs=4)
                    nc.vector.tensor_tensor(t2[:], rp[:], sinq, MULT)
                    nc.vector.tensor_tensor(dst, dst, t2[:], ADD)

                for h in range(GQ):
                    rope(qp[h], qt_sb[h][:, q0 : q0 + 512])
                rope(kp, kt_sb[:, q0 : q0 + 512])

                # V: evacuate V^T, then PE-transpose to natural [k, D] tiles
                vraw = ptmp.tile([P, 512], f32, name="vraw", tag="vraw")
                nc.scalar.copy(vraw[:], vp[:])
                for ks in range(4):
                    tp = aux_ps.tile([P, P], f32, name="vtrp", tag="aux")
                    nc.tensor.transpose(tp[:], vraw[:, ks * P : (ks + 1) * P], ident[:])
                    nc.vector.tensor_copy(v_sb[:, qc * 4 + ks, :], tp[:])

        # -------- phase 2: causal attention + interleaved o_proj --------
        with (
            tc.tile_pool(name="wo_pool", bufs=1) as wo_pool,
            tc.tile_pool(name="pt_pool", bufs=3) as pt_pool,
            tc.tile_pool(name="pair_pool", bufs=3) as pair_pool,
            tc.tile_pool(name="s_ps", bufs=2, space="PSUM") as s_ps,
            tc.tile_pool(name="y_ps", bufs=2, space="PSUM") as y_ps,
            tc.tile_pool(name="ro_ps", bufs=2, space="PSUM") as ro_ps,
            tc.tile_pool(name="nrm", bufs=2) as nrm_pool,
            tc.tile_pool(name="ost", bufs=4) as ost_pool,
        ):
            # wo resident for all of phase 2 (32KB/partition); the per-slice
            # DMAs land during attention chunk 0 (subtile deps let o_proj
            # start as soon as its ct slice is in).
            wo_sb = wo_pool.tile([P, GQ, NCT, 512], f32r)
            for ct in range(NCT):
                for h in range(GQ):
                    q = nc.sync if (h % 2 == 0) else nc.gpsimd
                    q.dma_start(wo_sb[:, h, ct, :], wo_r[:, h, ct, :])
            for aq in range(NQC):
                q0 = aq * 512
                nks = 4 * aq + 4  # number of 128-wide k subtiles (incl diag 4)
                npair = nks // 2
                for h in range(GQ):
                    qrhs = qt_sb[h][:, q0 : q0 + 512]
                    yp = y_ps.tile([P, 512], f32, name="yp", tag="yp")
                    rs = ro_ps.tile([P, 512], f32, name="rs", tag="ro")
                    for g in range(npair):
                        subs = (2 * g, 2 * g + 1)
                        sp = s_ps.tile([P, 1024], f32, name="sp", tag="sp")
                        pt = pt_pool.tile([P, 1024], f32r, name="ptile", tag="ptile")

                        # diagonal trim: subtile m only has valid q >= 128m;
                        # compute S/PV only there (capped so the fp32r matmul
                        # keeps >=256 moving rows). exp still covers the full
                        # tile; the mask multiply zeroes the stale region.
                        # aq==0 stays untrimmed so the first use of each PSUM
                        # slot writes the full width (exp reads stale PSUM
                        # in trimmed regions, which must be finite).
                        def off(ks):
                            m = ks - (nks - 4)
                            if aq == 0 or m <= 0:
                                return 0
                            return (0, 128, 256, 256)[m]

                        for j, ks in enumerate(subs):
                            o = off(ks)
                            nc.tensor.matmul(
                                sp[:, j * 512 + o : (j + 1) * 512],
                                kt_sb[:, ks * P : (ks + 1) * P],
                                qrhs[:, o:512],
                                start=True,
                                stop=True,
                            )
                        nc.scalar.activation(pt[:], sp[:], Exp, scale=SCALE)
                        for j, ks in enumerate(subs):
                            m = ks - (nks - 4)  # diagonal subtile index 0..3
                            if m >= 0:
                                w = 128 * (m + 1)
                                sl = pt[:, j * 512 : j * 512 + w]
                                nc.vector.tensor_tensor(
                                    sl, sl, mask_sb[:, m, :w], MULT
                                )
                        for j, ks in enumerate(subs):
                            first, last = ks == 0, ks == nks - 1
                            o = off(ks)
                            prhs = pt[:, j * 512 + o : (j + 1) * 512]
                            nc.tensor.matmul(
                                yp[:, o:512],
                                v_sb[:, ks, :],
                                prhs,
                                start=first,
                                stop=last,
                            )
                        # rowsum: pair-sum the two exp subtiles on DVE, then
                        # one ones-matmul per pair (half the PE rows). Both
                        # subtiles of a pair are valid from min(off) on
                        # (masked to zero below that).
                        po = min(off(subs[0]), off(subs[1]))
                        pair = pair_pool.tile([P, 512], f32r, name="pair", tag="pair")
                        nc.vector.tensor_tensor(
                            pair[:, po:512],
                            pt[:, po:512],
                            pt[:, 512 + po : 1024],
                            ADD,
                        )
                        nc.tensor.matmul(
                            rs[:, po:512],
                            ones_sb[:],
                            pair[:, po:512],
                            start=(g == 0),
                            stop=(g == npair - 1),
                        )
                    # 1/rowsum: single custom-DVE op (~18 bits, plenty
                    # above the fp32r matmul noise floor; rowsum >= 1 so no
                    # edge cases).
                    rinv = nrm_pool.tile([P, 512], f32, name="rinv", tag="rinv")
                    nc.vector.reciprocal_approx_fast(rinv[:], rs[:])
                    nc.vector.tensor_tensor(
                        y_sb[h][:, q0 : q0 + 512], yp[:], rinv[:], MULT
                    )
                # o_proj for this 512-row chunk (all 4 heads' y ready);
                # op tiles share the 2-buffer "ro" PSUM rotation with rs.
                for ct in range(NCT):
                    for qb in range(4 * aq, 4 * aq + 4):
                        op = ro_ps.tile([P, 512], f32, name="op", tag="ro")
                        for h in range(GQ):
                            nc.tensor.matmul(
                                op[:],
                                y_sb[h][:, qb * P : (qb + 1) * P],
                                wo_sb[:, h, ct, :],
                                start=(h == 0),
                                stop=(h == GQ - 1),
                            )
                        ot = ost_pool.tile([P, 512], f32, name="ot", tag="ot")
                        nc.scalar.copy(ot[:], op[:])
                        oq = nc.gpsimd if (ct % 2 == 0) else nc.scalar
                        oq.dma_start(
                            out[qb * P : (qb + 1) * P, ct * 512 : (ct + 1) * 512],
                            ot[:],
                        )

    nc.compile()
    return nc


def make_in_maps(x, wq, wk, wv, wo, T=T_FULL):
    """Per-core input dicts for run_bass_kernel_spmd."""
    cosT, sinT = _rope_tables(T)
    m4 = _mask4()
    onesm = np.ones((P, P), dtype=np.float32)
    rotm = _rot_lhsT()

    xts = [np.ascontiguousarray(x[b].T.astype(np.float32)) for b in range(B)]
    in_maps = []
    for core in range(NCORES):
        b, g = core // 4, core % 4
        in_maps.append(
            {
                "xt": xts[b],
                "wq": np.ascontiguousarray(wq[:, 512 * g : 512 * (g + 1)]),
                "wk": np.ascontiguousarray(wk[:, D * g : D * (g + 1)]),
                "wv": np.ascontiguousarray(wv[:, D * g : D * (g + 1)]),
                "wo": np.ascontiguousarray(wo[512 * g : 512 * (g + 1), :]),
                "cosT": cosT,
                "sinT": sinT,
                "mask4": m4,
                "onesm": onesm,
                "rotm": rotm,
            }
        )
    return in_maps


_NC_CACHE = {}


def _get_nc(T=T_FULL):
    if T not in _NC_CACHE:
        _NC_CACHE[T] = build_nc(T)
    return _NC_CACHE[T]


def run(inputs, trace=False):
    """Run on 8 NeuronCores. Returns (full_output, BassKernelResults)."""
    from concourse.bass_utils import run_bass_kernel_spmd

    x = np.asarray(inputs["x"], dtype=np.float32)
    in_maps = make_in_maps(
        x,
        np.asarray(inputs["wq"], dtype=np.float32),
        np.asarray(inputs["wk"], dtype=np.float32),
        np.asarray(inputs["wv"], dtype=np.float32),
        np.asarray(inputs["wo"], dtype=np.float32),
    )
    nc = _get_nc()
    res = run_bass_kernel_spmd(nc, in_maps, list(range(NCORES)), trace=trace)
    outs = res.results
    full = np.zeros((B, T_FULL, C_DIM), dtype=np.float32)
    for core in range(NCORES):
        full[core // 4] += outs[core]["out"]
    return full, res


def kernel(**inputs):
    full, _ = run(inputs, trace=False)
    return full


# revision 15
# speedup vs baseline: 1.2287x; 1.0561x over previous
"""Trainium2 Bass kernel for causal self-attention with GQA + RoPE.

Model: B=2, T=2048, C=2048, H=16 query heads, H_KV=4 kv heads, D=128.

Sharding (8 NeuronCores, pure SPMD, no collectives):
  core i -> batch b = i // 4, kv-group g = i % 4
            (query heads 4g..4g+3, kv head g, all T positions of batch b).
  Every core runs an identical program; only input data differs.
  o_proj is computed against the row-slice wo[512g:512(g+1), :], giving a
  partial [T, C] output per core; the sum over the 4 cores of each batch
  (the tensor-parallel all-reduce) is done on the host in numpy.

Device program per core (all matmuls fp32r = full PE rate at N>=256):
  - activations kept transposed: Q^T/K^T are [D, T] (D on partitions), which
    is what both the projection matmuls and the S^T = K @ Q^T matmuls want.
  - V is produced as V^T [D, T] then PE-transposed into natural [T, D] tiles
    (lhsT for the PV matmul).
  - RoPE: rotate_half is the linear map R, applied as a PE matmul
    (lhsT = R^T), then q_rope = q * cos + (R q) * sin on the vector engine.
  - causal flash-style attention without row-max (logits are provably small
    for this problem: |s| < ~6, exp never overflows), q in 512-wide chunks:
       S^T[k, q] 512-wide subtiles -> exp(scale*s) on ACT (psum -> sbuf)
       -> causal mask multiply on the 4 diagonal subtiles (host masks)
       -> y^T accumulated via lhsT=V tiles
       -> rowsum: DVE pair-sums adjacent exp subtiles, then a single
          ones-lhsT matmul accumulates the pairs in PSUM (half the PE rows
          of the naive per-subtile ones matmul)
       -> 1/rowsum via DVE reciprocal_approx_fast, y = y * rinv on DVE.
  - o_proj per 512-row chunk; wo is loaded ONCE at phase-2 start and kept
    resident (32KB/partition) instead of being re-streamed per chunk.
  - xt streams in [P, 2, 512] tiles with bufs=8 so the next chunk's DMA
    issues ~20us ahead of use (the bufs=4 whole-chunk scheme stalled all
    engines ~5-7us at every chunk boundary).
  - o_proj PSUM evacuation on the ACT engine (DVE was near-critical);
    o_proj output tiles and rowsum tiles share a 2-buffer PSUM rotation.
"""

import math
import os

import numpy as np

os.environ.setdefault("MYCRO_LOCAL_CACHE", "1")

P = 128
D = 128
H = 16
H_KV = 4
GQ = H // H_KV  # 4 query heads per kv head (= per core)
B = 2
T_FULL = 2048
C_DIM = 2048
NCORES = 8
ROPE_BASE = 10000.0


def _rope_tables(T):
    inv_freq = 1.0 / (ROPE_BASE ** (np.arange(0, D, 2, dtype=np.float32) / D))
    t = np.arange(T, dtype=np.float32)
    freqs = np.outer(t, inv_freq)  # [T, D/2]
    emb = np.concatenate((freqs, freqs), axis=-1)  # [T, D]
    return (
        np.ascontiguousarray(np.cos(emb).T.astype(np.float32)),  # [D, T]
        np.ascontiguousarray(np.sin(emb).T.astype(np.float32)),
    )


def _rot_lhsT():
    # rotate_half(q) = R @ q with R[d, d+64] = -1 (d < 64), R[d, d-64] = +1.
    # matmul computes lhsT.T @ rhs, so pass lhsT = R^T.
    R = np.zeros((D, D), dtype=np.float32)
    half = D // 2
    R[np.arange(half), np.arange(half) + half] = -1.0
    R[np.arange(half) + half, np.arange(half)] = 1.0
    return np.ascontiguousarray(R.T)


def _mask4():
    # mask4[m][k, q] = 1 if (128*m + k) <= q else 0, for the 4 diagonal
    # k-subtiles of a 512-wide q chunk (S^T layout: k on partitions).
    m4 = np.zeros((4, P, 512), dtype=np.float32)
    q = np.arange(512)
    k = np.arange(P)
    for m in range(4):
        m4[m] = ((128 * m + k)[:, None] <= q[None, :]).astype(np.float32)
    return m4


def build_nc(T=T_FULL):
    """Build the per-core Bass/Tile program (identical across cores)."""
    from contextlib import ExitStack

    import concourse.mybir as mybir
    import concourse.tile as tile
    from concourse import bacc
    from concourse.masks import make_identity

    f32 = mybir.dt.float32
    f32r = mybir.dt.float32r
    bf16 = mybir.dt.bfloat16
    Exp = mybir.ActivationFunctionType.Exp
    MULT = mybir.AluOpType.mult
    ADD = mybir.AluOpType.add
    SCALE = 1.0 / math.sqrt(D)

    NCC = C_DIM // P  # 16 contraction chunks
    NQC = T // 512  # projection / attention q-chunks (512-wide)
    NCT = C_DIM // 512  # o_proj column tiles
    NKB = T // P  # 128-wide k subtiles
    XG = 2  # xt c-chunks per streamed tile

    nc = bacc.Bacc(
        "TRN2",
        target_bir_lowering=False,
        debug=False,
        num_devices=NCORES,
    )

    xt = nc.dram_tensor("xt", [C_DIM, T], f32r, kind="ExternalInput").ap()
    wq = nc.dram_tensor("wq", [C_DIM, GQ * D], f32r, kind="ExternalInput").ap()
    wk = nc.dram_tensor("wk", [C_DIM, D], f32r, kind="ExternalInput").ap()
    wv = nc.dram_tensor("wv", [C_DIM, D], f32r, kind="ExternalInput").ap()
    wo = nc.dram_tensor("wo", [GQ * D, C_DIM], bf16, kind="ExternalInput").ap()
    cosT = nc.dram_tensor("cosT", [D, T], f32, kind="ExternalInput").ap()
    sinT = nc.dram_tensor("sinT", [D, T], f32, kind="ExternalInput").ap()
    mask4 = nc.dram_tensor("mask4", [4, P, 512], bf16, kind="ExternalInput").ap()
    onesm = nc.dram_tensor("onesm", [P, P], bf16, kind="ExternalInput").ap()
    rotm = nc.dram_tensor("rotm", [P, P], f32r, kind="ExternalInput").ap()
    out = nc.dram_tensor("out", [T, C_DIM], f32, kind="ExternalOutput").ap()

    with tile.TileContext(nc) as tc, ExitStack() as ctx:
        const = ctx.enter_context(tc.tile_pool(name="const", bufs=1))
        acts = ctx.enter_context(tc.tile_pool(name="acts", bufs=1))

        wq_r = wq.rearrange("(cc p) n -> p cc n", p=P)
        wk_r = wk.rearrange("(cc p) n -> p cc n", p=P)
        wv_r = wv.rearrange("(cc p) n -> p cc n", p=P)
        xt_r = xt.rearrange("(cc p) t -> p cc t", p=P)
        wo_r = wo.rearrange("(h p) (ct n) -> p h ct n", p=P, n=512)

        ones_sb = const.tile([P, P], bf16)
        rot_sb = const.tile([P, P], f32r)
        ident = const.tile([P, P], bf16)
        mask_sb = const.tile([P, 4, 512], bf16)

        # long-lived activations: everything consumed by phase-2 matmuls is
        # bf16 — non-fp32 operands re-enable the compiler's automatic fast
        # weight load (FWL) + LDWEIGHTS pull-ahead, which fp32r disables
        # (fp32_mode=HIGH guard), and bf16 gets the DVE 2x/4x perf modes.
        qt_sb = [acts.tile([P, T], bf16, name=f"qt{h}") for h in range(GQ)]
        kt_sb = acts.tile([P, T], bf16, name="kt")
        v_sb = acts.tile([P, NKB, D], bf16, name="vnat")
        y_sb = [acts.tile([P, T], bf16, name=f"yt{h}") for h in range(GQ)]

        # ---------------- phase 1: projections + rope ----------------
        NXT = NCC // XG  # xt tiles per q-chunk
        with (
            tc.tile_pool(name="pwts", bufs=1) as wpool,
            tc.tile_pool(name="xts", bufs=8) as xt_pool,
            tc.tile_pool(name="rope_t", bufs=1) as rope_pool,
            tc.tile_pool(name="proj_ps", bufs=1, space="PSUM") as proj_ps,
            tc.tile_pool(name="aux_ps", bufs=2, space="PSUM") as aux_ps,
            tc.tile_pool(name="ptmp", bufs=2) as ptmp,
        ):
            # weight tiles: per-cc DMAs so the first projection matmul can
            # start as soon as chunk 0 lands. xt streams on the sync queue;
            # wq on scalar; wk/wv + consts on gpsimd.
            wq_sb = wpool.tile([P, NCC, GQ * D], f32r)
            wk_sb = wpool.tile([P, NCC, D], f32r)
            wv_sb = wpool.tile([P, NCC, D], f32r)
            # first-needed first: wq/wk/wv chunk 0, lead xt tiles
            nc.scalar.dma_start(wq_sb[:, 0, :], wq_r[:, 0, :])
            nc.gpsimd.dma_start(wk_sb[:, 0, :], wk_r[:, 0, :])
            nc.gpsimd.dma_start(wv_sb[:, 0, :], wv_r[:, 0, :])
            lead_xs = []
            for xg in range(3):
                xs = xt_pool.tile([P, XG, 512], f32r, tag="xt", name=f"xs_l{xg}")
                nc.sync.dma_start(xs[:], xt_r[:, xg * XG : (xg + 1) * XG, 0:512])
                lead_xs.append(xs)
            nc.gpsimd.dma_start(rot_sb[:], rotm)
            # cos/sin on the fast sync HW queue, ahead of the chunk-1 xt
            # stream: on the gpsimd SW queue they landed ~60us in and stalled
            # the entire chunk-0 rope.
            cos_sb = rope_pool.tile([P, T], f32)
            nc.sync.dma_start(cos_sb[:], cosT)
            sin_sb = rope_pool.tile([P, T], f32)
            nc.sync.dma_start(sin_sb[:], sinT)
            for cc in range(1, NCC):
                nc.scalar.dma_start(wq_sb[:, cc, :], wq_r[:, cc, :])
                nc.gpsimd.dma_start(wk_sb[:, cc, :], wk_r[:, cc, :])
                nc.gpsimd.dma_start(wv_sb[:, cc, :], wv_r[:, cc, :])
            make_identity(nc, ident)
            nc.gpsimd.dma_start(ones_sb[:], onesm)
            for m in range(4):
                nc.gpsimd.dma_start(mask_sb[:, m, :], mask4[m])
            for qc in range(NQC):
                q0 = qc * 512
                xt_tiles = []
                for xg in range(NXT):
                    if qc == 0 and xg < 3:
                        xt_tiles.append(lead_xs[xg])
                        continue
                    xs = xt_pool.tile([P, XG, 512], f32r, tag="xt")
                    nc.sync.dma_start(
                        xs[:], xt_r[:, xg * XG : (xg + 1) * XG, q0 : q0 + 512]
                    )
                    xt_tiles.append(xs)

                qp = [
                    proj_ps.tile([P, 512], f32, name=f"qp{h}", tag=f"qp{h}")
                    for h in range(GQ)
                ]
                kp = proj_ps.tile([P, 512], f32, name="kp", tag="kp")
                vp = proj_ps.tile([P, 512], f32, name="vp", tag="vp")
                for cc in range(NCC):
                    xtile = xt_tiles[cc // XG][:, cc % XG, :]
                    first, last = cc == 0, cc == NCC - 1
                    for h in range(GQ):
                        nc.tensor.matmul(
                            qp[h][:],
                            wq_sb[:, cc, h * D : (h + 1) * D],
                            xtile,
                            start=first,
                            stop=last,
                        )
                    nc.tensor.matmul(
                        kp[:], wk_sb[:, cc, :], xtile, start=first, stop=last
                    )
                    nc.tensor.matmul(
                        vp[:], wv_sb[:, cc, :], xtile, start=first, stop=last
                    )

                cosq = cos_sb[:, q0 : q0 + 512]
                sinq = sin_sb[:, q0 : q0 + 512]

                def rope(pt_ps, dst):
                    # dst = pt*cos + (R pt)*sin ; pt_ps is the PSUM projection
                    raw = ptmp.tile([P, 512], f32r, name="rraw", tag="rraw", bufs=4)
                    nc.scalar.copy(raw[:], pt_ps[:])
                    rp = aux_ps.tile([P, 512], f32, name="rotp", tag="aux")
                    nc.tensor.matmul(rp[:], rot_sb[:], raw[:], start=True, stop=True)
                    nc.vector.tensor_tensor(dst, raw[:], cosq, MULT)
                    t2 = ptmp.tile([P, 512], f32, name="rt2", tag="rt2", bufs=4)
                    nc.vector.tensor_tensor(t2[:], rp[:], sinq, MULT)
                    nc.vector.tensor_tensor(dst, dst, t2[:], ADD)

                for h in range(GQ):
                    rope(qp[h], qt_sb[h][:, q0 : q0 + 512])
                rope(kp, kt_sb[:, q0 : q0 + 512])

                # V: evacuate V^T, then PE-transpose to natural [k, D] tiles
                vraw = ptmp.tile([P, 512], bf16, name="vraw", tag="vraw")
                nc.scalar.copy(vraw[:], vp[:])
                for ks in range(4):
                    tp = aux_ps.tile([P, P], bf16, name="vtrp", tag="aux")
                    nc.tensor.transpose(tp[:], vraw[:, ks * P : (ks + 1) * P], ident[:])
                    nc.vector.tensor_copy(v_sb[:, qc * 4 + ks, :], tp[:])

        # -------- phase 2: causal attention + interleaved o_proj --------
        with (
            tc.tile_pool(name="wo_pool", bufs=1) as wo_pool,
            tc.tile_pool(name="pt_pool", bufs=3) as pt_pool,
            tc.tile_pool(name="pair_pool", bufs=3) as pair_pool,
            tc.tile_pool(name="s_ps", bufs=2, space="PSUM") as s_ps,
            tc.tile_pool(name="y_ps", bufs=2, space="PSUM") as y_ps,
            tc.tile_pool(name="ro_ps", bufs=2, space="PSUM") as ro_ps,
            tc.tile_pool(name="nrm", bufs=2) as nrm_pool,
            tc.tile_pool(name="ost", bufs=4) as ost_pool,
        ):
            # wo resident for all of phase 2 (32KB/partition); the per-slice
            # DMAs land during attention chunk 0 (subtile deps let o_proj
            # start as soon as its ct slice is in).
            wo_sb = wo_pool.tile([P, GQ, NCT, 512], bf16)
            for ct in range(NCT):
                for h in range(GQ):
                    q = nc.sync if (h % 2 == 0) else nc.gpsimd
                    q.dma_start(wo_sb[:, h, ct, :], wo_r[:, h, ct, :])
            for aq in range(NQC):
                q0 = aq * 512
                nks = 4 * aq + 4  # number of 128-wide k subtiles (incl diag 4)
                npair = nks // 2
                for h in range(GQ):
                    qrhs = qt_sb[h][:, q0 : q0 + 512]
                    yp = y_ps.tile([P, 512], f32, name="yp", tag="yp")
                    rs = ro_ps.tile([P, 512], f32, name="rs", tag="ro")
                    for g in range(npair):
                        subs = (2 * g, 2 * g + 1)
                        sp = s_ps.tile([P, 1024], f32, name="sp", tag="sp")
                        pt = pt_pool.tile([P, 1024], bf16, name="ptile", tag="ptile")

                        # diagonal trim: subtile m only has valid q >= 128m;
                        # compute S/PV only there (capped so the fp32r matmul
                        # keeps >=256 moving rows). exp still covers the full
                        # tile; the mask multiply zeroes the stale region.
                        # aq==0 stays untrimmed so the first use of each PSUM
                        # slot writes the full width (exp reads stale PSUM
                        # in trimmed regions, which must be finite).
                        def off(ks):
                            m = ks - (nks - 4)
                            if aq == 0 or m <= 0:
                                return 0
                            return (0, 128, 256, 256)[m]

                        for j, ks in enumerate(subs):
                            o = off(ks)
                            nc.tensor.matmul(
                                sp[:, j * 512 + o : (j + 1) * 512],
                                kt_sb[:, ks * P : (ks + 1) * P],
                                qrhs[:, o:512],
                                start=True,
                                stop=True,
                            )
                        nc.scalar.activation(pt[:], sp[:], Exp, scale=SCALE)
                        for j, ks in enumerate(subs):
                            m = ks - (nks - 4)  # diagonal subtile index 0..3
                            if m >= 0:
                                w = 128 * (m + 1)
                                sl = pt[:, j * 512 : j * 512 + w]
                                nc.vector.tensor_tensor(
                                    sl, sl, mask_sb[:, m, :w], MULT
                                )
                        for j, ks in enumerate(subs):
                            first, last = ks == 0, ks == nks - 1
                            o = off(ks)
                            prhs = pt[:, j * 512 + o : (j + 1) * 512]
                            nc.tensor.matmul(
                                yp[:, o:512],
                                v_sb[:, ks, :],
                                prhs,
                                start=first,
                                stop=last,
                            )
                        # rowsum: pair-sum the two exp subtiles on DVE, then
                        # one ones-matmul per pair (half the PE rows). Both
                        # subtiles of a pair are valid from min(off) on
                        # (masked to zero below that).
                        po = min(off(subs[0]), off(subs[1]))
                        pair = pair_pool.tile([P, 512], bf16, name="pair", tag="pair")
                        nc.vector.tensor_tensor(
                            pair[:, po:512],
                            pt[:, po:512],
                            pt[:, 512 + po : 1024],
                            ADD,
                        )
                        nc.tensor.matmul(
                            rs[:, po:512],
                            ones_sb[:],
                            pair[:, po:512],
                            start=(g == 0),
                            stop=(g == npair - 1),
                        )
                    # 1/rowsum: single custom-DVE op (~18 bits, plenty
                    # above the fp32r matmul noise floor; rowsum >= 1 so no
                    # edge cases).
                    rinv = nrm_pool.tile([P, 512], f32, name="rinv", tag="rinv")
                    nc.vector.reciprocal_approx_fast(rinv[:], rs[:])
                    nc.vector.tensor_tensor(
                        y_sb[h][:, q0 : q0 + 512], yp[:], rinv[:], MULT
                    )
                # o_proj for this 512-row chunk (all 4 heads' y ready);
                # op tiles share the 2-buffer "ro" PSUM rotation with rs.
                for ct in range(NCT):
                    for qb in range(4 * aq, 4 * aq + 4):
                        op = ro_ps.tile([P, 512], f32, name="op", tag="ro")
                        for h in range(GQ):
                            nc.tensor.matmul(
                                op[:],
                                y_sb[h][:, qb * P : (qb + 1) * P],
                                wo_sb[:, h, ct, :],
                                start=(h == 0),
                                stop=(h == GQ - 1),
                            )
                        ot = ost_pool.tile([P, 512], f32, name="ot", tag="ot")
                        nc.scalar.copy(ot[:], op[:])
                        oq = nc.gpsimd if (ct % 2 == 0) else nc.scalar
                        oq.dma_start(
                            out[qb * P : (qb + 1) * P, ct * 512 : (ct + 1) * 512],
                            ot[:],
                        )

    nc.compile()
    return nc


def make_in_maps(x, wq, wk, wv, wo, T=T_FULL):
    """Per-core input dicts for run_bass_kernel_spmd."""
    import ml_dtypes

    bf16 = ml_dtypes.bfloat16
    cosT, sinT = _rope_tables(T)
    m4 = _mask4().astype(bf16)
    onesm = np.ones((P, P), dtype=bf16)
    rotm = _rot_lhsT()

    xts = [np.ascontiguousarray(x[b].T.astype(np.float32)) for b in range(B)]
    in_maps = []
    for core in range(NCORES):
        b, g = core // 4, core % 4
        in_maps.append(
            {
                "xt": xts[b],
                "wq": np.ascontiguousarray(wq[:, 512 * g : 512 * (g + 1)]),
                "wk": np.ascontiguousarray(wk[:, D * g : D * (g + 1)]),
                "wv": np.ascontiguousarray(wv[:, D * g : D * (g + 1)]),
                "wo": np.ascontiguousarray(wo[512 * g : 512 * (g + 1), :]).astype(
                    bf16
                ),
                "cosT": cosT,
                "sinT": sinT,
                "mask4": m4,
                "onesm": onesm,
                "rotm": rotm,
            }
        )
    return in_maps


_NC_CACHE = {}


def _get_nc(T=T_FULL):
    if T not in _NC_CACHE:
        _NC_CACHE[T] = build_nc(T)
    return _NC_CACHE[T]


def run(inputs, trace=False):
    """Run on 8 NeuronCores. Returns (full_output, BassKernelResults)."""
    from concourse.bass_utils import run_bass_kernel_spmd

    x = np.asarray(inputs["x"], dtype=np.float32)
    in_maps = make_in_maps(
        x,
        np.asarray(inputs["wq"], dtype=np.float32),
        np.asarray(inputs["wk"], dtype=np.float32),
        np.asarray(inputs["wv"], dtype=np.float32),
        np.asarray(inputs["wo"], dtype=np.float32),
    )
    nc = _get_nc()
    res = run_bass_kernel_spmd(nc, in_maps, list(range(NCORES)), trace=trace)
    outs = res.results
    full = np.zeros((B, T_FULL, C_DIM), dtype=np.float32)
    for core in range(NCORES):
        full[core // 4] += outs[core]["out"]
    return full, res


def kernel(**inputs):
    full, _ = run(inputs, trace=False)
    return full


# revision 24
# speedup vs baseline: 1.3581x; 1.1053x over previous
"""Trainium2 Bass kernel for causal self-attention with GQA + RoPE.

Model: B=2, T=2048, C=2048, H=16 query heads, H_KV=4 kv heads, D=128.

Sharding (8 NeuronCores, pure SPMD, no collectives):
  core i -> batch b = i // 4, kv-group g = i % 4
            (query heads 4g..4g+3, kv head g, all T positions of batch b).
  Every core runs an identical program; only input data differs.
  o_proj is computed against the row-slice wo[512g:512(g+1), :], giving a
  partial [T, C] output per core; the sum over the 4 cores of each batch
  (the tensor-parallel all-reduce) is done on the host in numpy.

Device program per core (all matmuls fp32r = full PE rate at N>=256):
  - activations kept transposed: Q^T/K^T are [D, T] (D on partitions), which
    is what both the projection matmuls and the S^T = K @ Q^T matmuls want.
  - V is produced as V^T [D, T] then PE-transposed into natural [T, D] tiles
    (lhsT for the PV matmul).
  - RoPE: rotate_half is the linear map R, applied as a PE matmul
    (lhsT = R^T), then q_rope = q * cos + (R q) * sin on the vector engine.
  - causal flash-style attention without row-max (logits are provably small
    for this problem: |s| < ~6, exp never overflows), q in 512-wide chunks:
       S^T[k, q] 512-wide subtiles -> exp(scale*s) on ACT (psum -> sbuf)
       -> causal mask multiply on the 4 diagonal subtiles (host masks)
       -> y^T accumulated via lhsT=V tiles
       -> rowsum: DVE pair-sums adjacent exp subtiles, then a single
          ones-lhsT matmul accumulates the pairs in PSUM (half the PE rows
          of the naive per-subtile ones matmul)
       -> 1/rowsum via DVE reciprocal_approx_fast, y = y * rinv on DVE.
  - o_proj per 512-row chunk; wo is loaded ONCE at phase-2 start and kept
    resident (32KB/partition) instead of being re-streamed per chunk.
  - xt streams in [P, 2, 512] tiles with bufs=8 so the next chunk's DMA
    issues ~20us ahead of use (the bufs=4 whole-chunk scheme stalled all
    engines ~5-7us at every chunk boundary).
  - o_proj PSUM evacuation on the ACT engine (DVE was near-critical);
    o_proj output tiles and rowsum tiles share a 2-buffer PSUM rotation.
"""

import math
import os

import numpy as np

os.environ.setdefault("MYCRO_LOCAL_CACHE", "1")

P = 128
D = 128
H = 16
H_KV = 4
GQ = H // H_KV  # 4 query heads per kv head (= per core)
B = 2
T_FULL = 2048
C_DIM = 2048
NCORES = 8
ROPE_BASE = 10000.0


def _rope_tables(T):
    inv_freq = 1.0 / (ROPE_BASE ** (np.arange(0, D, 2, dtype=np.float32) / D))
    t = np.arange(T, dtype=np.float32)
    freqs = np.outer(t, inv_freq)  # [T, D/2]
    emb = np.concatenate((freqs, freqs), axis=-1)  # [T, D]
    return (
        np.ascontiguousarray(np.cos(emb).T.astype(np.float32)),  # [D, T]
        np.ascontiguousarray(np.sin(emb).T.astype(np.float32)),
    )


def _rot_lhsT():
    # rotate_half(q) = R @ q with R[d, d+64] = -1 (d < 64), R[d, d-64] = +1.
    # matmul computes lhsT.T @ rhs, so pass lhsT = R^T.
    R = np.zeros((D, D), dtype=np.float32)
    half = D // 2
    R[np.arange(half), np.arange(half) + half] = -1.0
    R[np.arange(half) + half, np.arange(half)] = 1.0
    return np.ascontiguousarray(R.T)


def _mask4():
    # mask4[m][k, q] = 1 if (128*m + k) <= q else 0, for the 4 diagonal
    # k-subtiles of a 512-wide q chunk (S^T layout: k on partitions).
    m4 = np.zeros((4, P, 512), dtype=np.float32)
    q = np.arange(512)
    k = np.arange(P)
    for m in range(4):
        m4[m] = ((128 * m + k)[:, None] <= q[None, :]).astype(np.float32)
    return m4


def build_nc(T=T_FULL):
    """Build the per-core Bass/Tile program (identical across cores)."""
    from contextlib import ExitStack

    import concourse.mybir as mybir
    import concourse.tile as tile
    from concourse import bacc
    from concourse.masks import make_identity

    f32 = mybir.dt.float32
    f32r = mybir.dt.float32r
    bf16 = mybir.dt.bfloat16
    Exp = mybir.ActivationFunctionType.Exp
    MULT = mybir.AluOpType.mult
    ADD = mybir.AluOpType.add
    SCALE = 1.0 / math.sqrt(D)

    NCC = C_DIM // P  # 16 contraction chunks
    NQC = T // 512  # projection / attention q-chunks (512-wide)
    NCT = C_DIM // 512  # o_proj column tiles
    NKB = T // P  # 128-wide k subtiles
    XG = 2  # xt c-chunks per streamed tile

    nc = bacc.Bacc(
        "TRN2",
        target_bir_lowering=False,
        debug=False,
        num_devices=NCORES,
    )

    xt = nc.dram_tensor("xt", [C_DIM, T], bf16, kind="ExternalInput").ap()
    wq = nc.dram_tensor("wq", [C_DIM, GQ * D], bf16, kind="ExternalInput").ap()
    wk = nc.dram_tensor("wk", [C_DIM, D], bf16, kind="ExternalInput").ap()
    wv = nc.dram_tensor("wv", [C_DIM, D], bf16, kind="ExternalInput").ap()
    wo = nc.dram_tensor("wo", [GQ * D, C_DIM], bf16, kind="ExternalInput").ap()
    cosT = nc.dram_tensor("cosT", [D, T], bf16, kind="ExternalInput").ap()
    sinT = nc.dram_tensor("sinT", [D, T], bf16, kind="ExternalInput").ap()
    mask4 = nc.dram_tensor("mask4", [4, P, 512], bf16, kind="ExternalInput").ap()
    onesm = nc.dram_tensor("onesm", [P, P], bf16, kind="ExternalInput").ap()
    rotm = nc.dram_tensor("rotm", [P, P], bf16, kind="ExternalInput").ap()
    out = nc.dram_tensor("out", [T, C_DIM], f32, kind="ExternalOutput").ap()

    with tile.TileContext(nc) as tc, ExitStack() as ctx:
        const = ctx.enter_context(tc.tile_pool(name="const", bufs=1))
        acts = ctx.enter_context(tc.tile_pool(name="acts", bufs=1))

        wq_r = wq.rearrange("(cc p) n -> p cc n", p=P)
        wk_r = wk.rearrange("(cc p) n -> p cc n", p=P)
        wv_r = wv.rearrange("(cc p) n -> p cc n", p=P)
        xt_r = xt.rearrange("(cc p) t -> p cc t", p=P)
        wo_r = wo.rearrange("(h p) (ct n) -> p h ct n", p=P, n=512)

        ones_sb = const.tile([P, P], bf16)
        rot_sb = const.tile([P, P], bf16)
        ident = const.tile([P, P], bf16)
        mask_sb = const.tile([P, 4, 512], bf16)

        # long-lived activations: everything consumed by phase-2 matmuls is
        # bf16 — non-fp32 operands re-enable the compiler's automatic fast
        # weight load (FWL) + LDWEIGHTS pull-ahead, which fp32r disables
        # (fp32_mode=HIGH guard), and bf16 gets the DVE 2x/4x perf modes.
        qt_sb = [acts.tile([P, T], bf16, name=f"qt{h}") for h in range(GQ)]
        kt_sb = acts.tile([P, T], bf16, name="kt")
        v_sb = acts.tile([P, NKB, D], bf16, name="vnat")
        y_sb = [acts.tile([P, T], bf16, name=f"yt{h}") for h in range(GQ)]

        # wo is resident for the whole kernel (16KB/partition in bf16); its
        # DMAs go on the scalar HW queue behind wq so it's fully in SBUF by
        # the time o_proj starts.
        wo_pool = ctx.enter_context(tc.tile_pool(name="wo_pool", bufs=1))
        wo_sb = wo_pool.tile([P, GQ, NCT, 512], bf16)

        # ---------------- phase 1: projections + rope ----------------
        NXT = NCC // XG  # xt tiles per q-chunk
        with (
            tc.tile_pool(name="pwts", bufs=1) as wpool,
            tc.tile_pool(name="xts", bufs=10) as xt_pool,
            tc.tile_pool(name="rope_t", bufs=1) as rope_pool,
            tc.tile_pool(name="proj_ps", bufs=1, space="PSUM") as proj_ps,
            tc.tile_pool(name="aux_ps", bufs=2, space="PSUM") as aux_ps,
            tc.tile_pool(name="ptmp", bufs=2) as ptmp,
        ):
            # weight tiles: per-cc DMAs so the first projection matmul can
            # start as soon as chunk 0 lands. xt streams on the sync queue;
            # wq/wo on scalar; wk/wv + consts on gpsimd.
            wq_sb = wpool.tile([P, NCC, GQ * D], bf16)
            wk_sb = wpool.tile([P, NCC, D], bf16)
            wv_sb = wpool.tile([P, NCC, D], bf16)
            # first-needed first: wq/wk/wv chunk 0, lead xt tiles
            nc.scalar.dma_start(wq_sb[:, 0, :], wq_r[:, 0, :])
            nc.gpsimd.dma_start(wk_sb[:, 0, :], wk_r[:, 0, :])
            nc.gpsimd.dma_start(wv_sb[:, 0, :], wv_r[:, 0, :])
            lead_xs = []
            for xg in range(3):
                xs = xt_pool.tile([P, XG, 512], bf16, tag="xt", name=f"xs_l{xg}")
                nc.sync.dma_start(xs[:], xt_r[:, xg * XG : (xg + 1) * XG, 0:512])
                lead_xs.append(xs)
            nc.gpsimd.dma_start(rot_sb[:], rotm)
            # cos/sin on the fast sync HW queue, ahead of the chunk-1 xt
            # stream: on the gpsimd SW queue they landed ~60us in and stalled
            # the entire chunk-0 rope.
            cos_sb = rope_pool.tile([P, T], bf16)
            nc.sync.dma_start(cos_sb[:], cosT)
            sin_sb = rope_pool.tile([P, T], bf16)
            nc.sync.dma_start(sin_sb[:], sinT)
            for cc in range(1, NCC):
                nc.scalar.dma_start(wq_sb[:, cc, :], wq_r[:, cc, :])
                nc.gpsimd.dma_start(wk_sb[:, cc, :], wk_r[:, cc, :])
                nc.gpsimd.dma_start(wv_sb[:, cc, :], wv_r[:, cc, :])
            for ct in range(NCT):
                for h in range(GQ):
                    nc.scalar.dma_start(wo_sb[:, h, ct, :], wo_r[:, h, ct, :])
            make_identity(nc, ident)
            nc.gpsimd.dma_start(ones_sb[:], onesm)
            for m in range(4):
                nc.gpsimd.dma_start(mask_sb[:, m, :], mask4[m])
            for qc in range(NQC):
                q0 = qc * 512
                xt_tiles = []
                for xg in range(NXT):
                    if qc == 0 and xg < 3:
                        xt_tiles.append(lead_xs[xg])
                        continue
                    xs = xt_pool.tile([P, XG, 512], bf16, tag="xt")
                    nc.sync.dma_start(
                        xs[:], xt_r[:, xg * XG : (xg + 1) * XG, q0 : q0 + 512]
                    )
                    xt_tiles.append(xs)

                qp = [
                    proj_ps.tile([P, 512], f32, name=f"qp{h}", tag=f"qp{h}")
                    for h in range(GQ)
                ]
                kp = proj_ps.tile([P, 512], f32, name="kp", tag="kp")
                vp = proj_ps.tile([P, 512], f32, name="vp", tag="vp")
                for cc in range(NCC):
                    xtile = xt_tiles[cc // XG][:, cc % XG, :]
                    first, last = cc == 0, cc == NCC - 1
                    for h in range(GQ):
                        nc.tensor.matmul(
                            qp[h][:],
                            wq_sb[:, cc, h * D : (h + 1) * D],
                            xtile,
                            start=first,
                            stop=last,
                        )
                    nc.tensor.matmul(
                        kp[:], wk_sb[:, cc, :], xtile, start=first, stop=last
                    )
                    nc.tensor.matmul(
                        vp[:], wv_sb[:, cc, :], xtile, start=first, stop=last
                    )

                cosq = cos_sb[:, q0 : q0 + 512]
                sinq = sin_sb[:, q0 : q0 + 512]

                def rope(pt_ps, dst):
                    # dst = pt*cos + (R pt)*sin ; pt_ps is the PSUM projection
                    raw = ptmp.tile([P, 512], bf16, name="rraw", tag="rraw", bufs=4)
                    nc.scalar.copy(raw[:], pt_ps[:])
                    rp = aux_ps.tile([P, 512], f32, name="rotp", tag="aux")
                    nc.tensor.matmul(rp[:], rot_sb[:], raw[:], start=True, stop=True)
                    nc.vector.tensor_tensor(dst, raw[:], cosq, MULT)
                    t2 = ptmp.tile([P, 512], bf16, name="rt2", tag="rt2", bufs=4)
                    nc.vector.tensor_tensor(t2[:], rp[:], sinq, MULT)
                    nc.vector.tensor_tensor(dst, dst, t2[:], ADD)

                for h in range(GQ):
                    rope(qp[h], qt_sb[h][:, q0 : q0 + 512])
                rope(kp, kt_sb[:, q0 : q0 + 512])

                # V: evacuate V^T, then PE-transpose to natural [k, D] tiles
                vraw = ptmp.tile([P, 512], bf16, name="vraw", tag="vraw")
                nc.scalar.copy(vraw[:], vp[:])
                for ks in range(4):
                    tp = aux_ps.tile([P, P], bf16, name="vtrp", tag="aux")
                    nc.tensor.transpose(tp[:], vraw[:, ks * P : (ks + 1) * P], ident[:])
                    nc.vector.tensor_copy(v_sb[:, qc * 4 + ks, :], tp[:])

        # -------- phase 2: causal attention + interleaved o_proj --------
        with (
            tc.tile_pool(name="pt_pool", bufs=4) as pt_pool,
            tc.tile_pool(name="pair_pool", bufs=4) as pair_pool,
            tc.tile_pool(name="s_ps", bufs=2, space="PSUM") as s_ps,
            tc.tile_pool(name="y_ps", bufs=2, space="PSUM") as y_ps,
            tc.tile_pool(name="ro_ps", bufs=2, space="PSUM") as ro_ps,
            tc.tile_pool(name="nrm", bufs=2) as nrm_pool,
            tc.tile_pool(name="ost", bufs=6) as ost_pool,
        ):
            for aq in range(NQC):
                q0 = aq * 512
                nks = 4 * aq + 4  # number of 128-wide k subtiles (incl diag 4)
                npair = nks // 2
                for h in range(GQ):
                    qrhs = qt_sb[h][:, q0 : q0 + 512]
                    yp = y_ps.tile([P, 512], f32, name="yp", tag="yp")
                    rs = ro_ps.tile([P, 512], f32, name="rs", tag="ro")
                    for g in range(npair):
                        subs = (2 * g, 2 * g + 1)
                        sp = s_ps.tile([P, 1024], f32, name="sp", tag="sp")
                        pt = pt_pool.tile([P, 1024], bf16, name="ptile", tag="ptile")

                        # diagonal trim: subtile m only has valid q >= 128m;
                        # compute S/PV only there (capped so the fp32r matmul
                        # keeps >=256 moving rows). exp still covers the full
                        # tile; the mask multiply zeroes the stale region.
                        # aq==0 stays untrimmed so the first use of each PSUM
                        # slot writes the full width (exp reads stale PSUM
                        # in trimmed regions, which must be finite).
                        def off(ks):
                            m = ks - (nks - 4)
                            if aq == 0 or m <= 0:
                                return 0
                            return (0, 128, 256, 256)[m]

                        for j, ks in enumerate(subs):
                            o = off(ks)
                            nc.tensor.matmul(
                                sp[:, j * 512 + o : (j + 1) * 512],
                                kt_sb[:, ks * P : (ks + 1) * P],
                                qrhs[:, o:512],
                                start=True,
                                stop=True,
                            )
                        e0 = off(subs[0])  # exp skips columns both subtiles lack
                        nc.scalar.activation(
                            pt[:, e0:1024], sp[:, e0:1024], Exp, scale=SCALE
                        )
                        for j, ks in enumerate(subs):
                            m = ks - (nks - 4)  # diagonal subtile index 0..3
                            if m >= 0:
                                w = 128 * (m + 1)
                                sl = pt[:, j * 512 : j * 512 + w]
                                nc.vector.tensor_tensor(
                                    sl, sl, mask_sb[:, m, :w], MULT
                                )
                        for j, ks in enumerate(subs):
                            first, last = ks == 0, ks == nks - 1
                            o = off(ks)
                            prhs = pt[:, j * 512 + o : (j + 1) * 512]
                            nc.tensor.matmul(
                                yp[:, o:512],
                                v_sb[:, ks, :],
                                prhs,
                                start=first,
                                stop=last,
                            )
                        # rowsum: pair-sum the two exp subtiles on DVE, then
                        # one ones-matmul per pair (half the PE rows). Both
                        # subtiles of a pair are valid from min(off) on
                        # (masked to zero below that).
                        po = min(off(subs[0]), off(subs[1]))
                        pair = pair_pool.tile([P, 512], bf16, name="pair", tag="pair")
                        nc.vector.tensor_tensor(
                            pair[:, po:512],
                            pt[:, po:512],
                            pt[:, 512 + po : 1024],
                            ADD,
                        )
                        nc.tensor.matmul(
                            rs[:, po:512],
                            ones_sb[:],
                            pair[:, po:512],
                            start=(g == 0),
                            stop=(g == npair - 1),
                        )
                    # 1/rowsum: single custom-DVE op (~18 bits, plenty
                    # above the fp32r matmul noise floor; rowsum >= 1 so no
                    # edge cases).
                    rinv = nrm_pool.tile([P, 512], f32, name="rinv", tag="rinv")
                    nc.vector.reciprocal_approx_fast(rinv[:], rs[:])
                    nc.vector.tensor_tensor(
                        y_sb[h][:, q0 : q0 + 512], yp[:], rinv[:], MULT
                    )
                # o_proj for this 512-row chunk (all 4 heads' y ready);
                # op tiles share the 2-buffer "ro" PSUM rotation with rs.
                for ct in range(NCT):
                    for qb in range(4 * aq, 4 * aq + 4):
                        op = ro_ps.tile([P, 512], f32, name="op", tag="ro")
                        for h in range(GQ):
                            nc.tensor.matmul(
                                op[:],
                                y_sb[h][:, qb * P : (qb + 1) * P],
                                wo_sb[:, h, ct, :],
                                start=(h == 0),
                                stop=(h == GQ - 1),
                            )
                        ot = ost_pool.tile([P, 512], f32, name="ot", tag="ot")
                        # alternate the PSUM evacuation between ACT and DVE so
                        # neither engine becomes the phase-2 bottleneck
                        if qb % 2 == 0:
                            nc.scalar.copy(ot[:], op[:])
                        else:
                            nc.vector.tensor_copy(ot[:], op[:])
                        oq = nc.gpsimd if (ct % 2 == 0) else nc.sync
                        oq.dma_start(
                            out[qb * P : (qb + 1) * P, ct * 512 : (ct + 1) * 512],
                            ot[:],
                        )

    nc.compile()
    return nc


def make_in_maps(x, wq, wk, wv, wo, T=T_FULL):
    """Per-core input dicts for run_bass_kernel_spmd."""
    import ml_dtypes

    bf16 = ml_dtypes.bfloat16
    cosT, sinT = _rope_tables(T)
    m4 = _mask4().astype(bf16)
    onesm = np.ones((P, P), dtype=bf16)
    rotm = _rot_lhsT().astype(bf16)

    xts = [np.ascontiguousarray(x[b].T).astype(bf16) for b in range(B)]
    in_maps = []
    for core in range(NCORES):
        b, g = core // 4, core % 4
        in_maps.append(
            {
                "xt": xts[b],
                "wq": np.ascontiguousarray(wq[:, 512 * g : 512 * (g + 1)]).astype(
                    bf16
                ),
                "wk": np.ascontiguousarray(wk[:, D * g : D * (g + 1)]).astype(bf16),
                "wv": np.ascontiguousarray(wv[:, D * g : D * (g + 1)]).astype(bf16),
                "wo": np.ascontiguousarray(wo[512 * g : 512 * (g + 1), :]).astype(
                    bf16
                ),
                "cosT": cosT.astype(bf16),
                "sinT": sinT.astype(bf16),
                "mask4": m4,
                "onesm": onesm,
                "rotm": rotm,
            }
        )
    return in_maps


_NC_CACHE = {}


def _get_nc(T=T_FULL):
    if T not in _NC_CACHE:
        _NC_CACHE[T] = build_nc(T)
    return _NC_CACHE[T]


def run(inputs, trace=False):
    """Run on 8 NeuronCores. Returns (full_output, BassKernelResults)."""
    from concourse.bass_utils import run_bass_kernel_spmd

    x = np.asarray(inputs["x"], dtype=np.float32)
    in_maps = make_in_maps(
        x,
        np.asarray(inputs["wq"], dtype=np.float32),
        np.asarray(inputs["wk"], dtype=np.float32),
        np.asarray(inputs["wv"], dtype=np.float32),
        np.asarray(inputs["wo"], dtype=np.float32),
    )
    nc = _get_nc()
    res = run_bass_kernel_spmd(nc, in_maps, list(range(NCORES)), trace=trace)
    outs = res.results
    full = np.zeros((B, T_FULL, C_DIM), dtype=np.float32)
    for core in range(NCORES):
        full[core // 4] += outs[core]["out"]
    return full, res


def kernel(**inputs):
    full, _ = run(inputs, trace=False)
    return full
